# revision 46
# baseline (speedup 1.0000x reference)
"""Trainium2 Bass kernel for nn_Block_16441134809284 (sparse_attention block).

Self-contained: token-parallel over 8 NeuronCores (2 batches x 4 slices of 512
tokens). Each core computes its 512 output tokens end-to-end; KV for the
sliding window is recomputed per core from a zero-padded token window, so no
collectives are needed.

v3 design notes (on top of v2):
- Software-pipelined segment schedule: K/V projection matmuls + stats of
  segment s+1 are emitted before the attends of segment s, and the
  normalize/rope (which gates on the per-segment stats Ln/Exp) is emitted
  after them, so the PE never queues behind the rstd chain.
- attend processes all 8 heads with a one-head lookahead (logits/softcap of
  head n before the AV matmuls of head n-1), and the final segment's
  attends interleave the per-head 1/softmax-sum normalize (PE broadcast +
  fast-approx DVE reciprocal) so attn_vec starts while later heads attend.
- Chunk-granular triangular skip: per key-chunk query ranges implied by the
  sliding window + causality cut ~25% of the logits/softcap/AV work; the
  first processed range covers the union so the PSUM `start` zeroing stays
  valid.
- The pre-attention rms_norm(x) is absorbed by the q/k/v rms_norms (all three
  normalize per token after the projection, and rms_norm is scale-invariant),
  so q/k/v are projected directly from x with (1+pre_attn_scale) folded into
  the weights.
- All large tensors ship as bf16, pre-tiled partition-major so every DMA is
  128 contiguous per-partition chunks (large descriptors). Activations stay
  fp32 (float32r) except where noted; matmuls mix bf16 weights (stationary)
  with f32r moving operands, which the PE allows.
- rstd = exp(-0.5*ln(mean+eps)) on ScalarE: avoids the banned Rsqrt, the slow
  DVE reciprocal, and keeps the attention phase within two ACT table sets
  (exp_and_others / natural_log_exp_and_others).
- Softmax without max-subtraction (logits are soft-capped to +-50); masking is
  a 0/1 multiplier on exp values.
"""
import sys

for _p in ("/opt/trn_rl_repo", "/root/.axon_site/_ro/trn_rl_repo"):
    if _p not in sys.path:
        sys.path.insert(0, _p)

import numpy as np
import ml_dtypes

BF = ml_dtypes.bfloat16

K_MASK = -2.3819763e+38
SOFT_CAP = 50.0
WINDOW = 1024
ROPE_BASE = 10000.0

B, T, D, N, KH, H, F = 2, 2048, 2048, 8, 4, 256, 8192
CACHE = 4096
N_CORES = 8
SLICES = N_CORES // B
TQ = T // SLICES          # 512
EPS = 1e-6
CH = 128                  # chunk (partition) size
DCH = D // CH             # 16
HCH = H // CH             # 2
SEG_CH = 4                # kv chunks per attention segment (512 tokens)
SEG = SEG_CH * CH         # 512
FPG = 8                   # F-chunks per FFN group
FGROUPS = F // CH // FPG  # 8
FHALF = FGROUPS // 2      # 4
G = N // KH               # 2 query heads per kv head


# ----------------------------------------------------------------------------
# host-side planning
# ----------------------------------------------------------------------------

def _plan(inputs):
    attn_mask = np.asarray(inputs['attn_mask'])
    seg_pos = np.asarray(inputs['segment_pos']).astype(np.int64)
    cache_pos_in = np.asarray(inputs['cache_positions']).astype(np.int64)
    end_index = np.asarray(inputs['end_index']).astype(np.int64)
    x = np.asarray(inputs['x'], dtype=np.float32)

    slot_of_tok = (end_index[:, None] + np.arange(T)[None, :]) % CACHE
    old_slots = (end_index[:, None] + T + np.arange(CACHE - T)[None, :]) % CACHE

    cache_pos = cache_pos_in.copy()
    bidx = np.arange(B)[:, None]
    cache_pos[bidx, slot_of_tok] = seg_pos
    sliding = (cache_pos[:, None, :] > seg_pos[:, :, None] - WINDOW) & \
              (cache_pos[:, None, :] < seg_pos[:, :, None] + WINDOW)
    mask = attn_mask & sliding                      # [B, T(query), S(slot)]

    mask_tok = np.take_along_axis(mask, slot_of_tok[:, None, :], axis=2)
    mask_old = np.take_along_axis(mask, old_slots[:, None, :], axis=2)

    L_need = R_need = 0
    cache_chunks_needed = 0
    n_old = CACHE - T
    for b in range(B):
        for s in range(SLICES):
            t0 = s * TQ
            used = mask_tok[b, t0:t0 + TQ].any(axis=0)
            cidx = np.nonzero(used.reshape(T // CH, CH).any(axis=1))[0]
            if len(cidx):
                L_need = max(L_need, t0 // CH - int(cidx[0]))
                R_need = max(R_need, int(cidx[-1]) + 1 - (t0 + TQ) // CH)
            used_o = mask_old[b, t0:t0 + TQ].any(axis=0)
            co = np.nonzero(used_o.reshape(n_old // CH, CH).any(axis=1))[0]
            cache_chunks_needed = max(cache_chunks_needed, len(co))

    L_need = ((max(L_need, 0) + SEG_CH - 1) // SEG_CH) * SEG_CH
    R_need = max(R_need, 0)
    W = L_need + TQ // CH + R_need
    W = ((W + SEG_CH - 1) // SEG_CH) * SEG_CH
    OWN_OFF = L_need * CH
    KV_TOK = W * CH
    NSEG = W // SEG_CH
    EXTRA = ((cache_chunks_needed + SEG_CH - 1) // SEG_CH) * SEG_CH \
        if cache_chunks_needed else 0

    per_core = []
    frac = 2.0 * np.arange(H // 2, dtype=np.float32) / np.float32(H)
    timescale = np.float32(ROPE_BASE) ** frac
    for c in range(N_CORES):
        b, s = divmod(c, SLICES)
        t0 = s * TQ
        toks = np.arange(t0 - OWN_OFF, t0 - OWN_OFF + KV_TOK)
        valid = (toks >= 0) & (toks < T)
        tv = np.clip(toks, 0, T - 1)

        xw = np.where(valid[:, None], x[b, tv], 0.0).astype(np.float32)
        # [NSEG, 128, DCH, SEG]: (s, p, dc, j) = xw[s*SEG + j, dc*128 + p]
        x_t = np.ascontiguousarray(
            xw.reshape(NSEG, SEG, DCH, CH).transpose(0, 3, 2, 1)).astype(np.float16)

        pos = np.where(valid, seg_pos[b, tv], 0).astype(np.float32)
        ang = pos[None, :] / timescale[:, None]
        sc = np.empty((CH, 2, KV_TOK), np.float32)
        sc[:, 0, :] = np.cos(ang)
        sc[:, 1, :] = np.sin(ang)
        sc_t = np.ascontiguousarray(sc).astype(np.float16)

        mb = mask_tok[b, t0:t0 + TQ][:, tv] & valid[None, :]
        bias_c = np.where(mb.T, 1.0, 0.0).astype(np.float32)  # [KV_TOK, TQ]
        # [NSEG, 128, SEG_CH, TQ]: (s, p, c, t) = bias_c[s*SEG + c*128 + p, t]
        bias_t = np.ascontiguousarray(
            bias_c.reshape(NSEG, SEG_CH, CH, TQ).transpose(0, 2, 1, 3)).astype(BF)

        entry = dict(b=b, t0=t0, x_t=x_t, sc_t=sc_t, bias_t=bias_t)

        if EXTRA:
            n_ex = EXTRA * CH
            mo = mask_old[b, t0:t0 + TQ]
            used_o = mo.any(axis=0)
            order = np.argsort(~used_o, kind='stable')
            sel = order[:n_ex]
            ck = np.asarray(inputs['cache_k'], dtype=np.float32)[b][old_slots[b][sel]]
            cv = np.asarray(inputs['cache_v'], dtype=np.float32)[b][old_slots[b][sel]]
            # kc: [KH, 128, HCH, n_ex]: (kh, p, hc, s) = ck[s, kh, hc*128+p]
            entry['kc_t'] = np.ascontiguousarray(
                ck.reshape(n_ex, KH, HCH, CH).transpose(1, 3, 2, 0)).astype(np.float16)
            # vc: [KH, EXTRA//SEG_CH, 128, SEG_CH, H]:
            #   (kh, sx, p, st, h) = cv[sx*SEG + st*128 + p, kh, h]
            entry['vc_t'] = np.ascontiguousarray(
                cv.reshape(EXTRA // SEG_CH, SEG_CH, CH, KH, H)
                .transpose(3, 0, 2, 1, 4)).astype(BF)
            bc_ = np.where(mo[:, sel].T, 1.0, 0.0).astype(np.float32)  # [n_ex, TQ]
            entry['biasc_t'] = np.ascontiguousarray(
                bc_.reshape(EXTRA // SEG_CH, SEG_CH, CH, TQ)
                .transpose(0, 2, 1, 3)).astype(BF)
        per_core.append(entry)

    return dict(W=W, OWN_OFF=OWN_OFF, KV_TOK=KV_TOK, NSEG=NSEG, EXTRA=EXTRA,
                s_x=1.0, per_core=per_core)


def _prep_weights(inputs, s_x=None):
    w_kv = np.asarray(inputs['w_kv'], dtype=np.float32)
    pre_attn = (1.0 + np.asarray(inputs['pre_attn_scale'], dtype=np.float32))
    pre_ffw = (1.0 + np.asarray(inputs['pre_ffw_scale'], dtype=np.float32))

    def tile_dh(w):  # [*, D, H] -> [*, 128, DCH, H]
        lead = w.shape[:-2]
        return np.ascontiguousarray(
            w.reshape(*lead, DCH, CH, H).transpose(
                *range(len(lead)), len(lead) + 1, len(lead), len(lead) + 2))

    wq = np.asarray(inputs['w_q'], dtype=np.float32) * pre_attn[None, :, None]
    wk = w_kv[0] * pre_attn[None, :, None]
    wv = w_kv[1] * pre_attn[None, :, None]

    wav = np.asarray(inputs['w_attn_vec'], dtype=np.float32)  # [N, H, D]
    # [N, 4, 128, HCH, 512]: (n, q4, p, hc, j) = wav[n, hc*128+p, q4*512+j]
    wav_t = np.ascontiguousarray(
        wav.reshape(N, HCH, CH, 4, TQ).transpose(0, 3, 2, 1, 4)).astype(BF)

    w_g = np.asarray(inputs['w_gating'], dtype=np.float32)    # [2, F, D]
    w_g_T = w_g.transpose(0, 2, 1) * pre_ffw[None, :, None]   # [2, D, F]
    # [2, 64, 128, DCH, 128]: (g, fc, p, dc, j) = w_g_T[g, dc*128+p, fc*128+j]
    wg_t = np.ascontiguousarray(
        w_g_T.reshape(2, DCH, CH, F // CH, CH).transpose(0, 3, 2, 1, 4)).astype(BF)

    w_lin = np.asarray(inputs['w_linear'], dtype=np.float32)  # [F, D]
    # [8, 4, 128, 8, 4, 128]:
    #   (fg, dcq, p, fc, dcl, j) = w_lin[(fg*8+fc)*128+p, (dcq*4+dcl)*128+j]
    wl_t = np.ascontiguousarray(
        w_lin.reshape(FGROUPS, FPG, CH, 4, 4, CH)
        .transpose(0, 3, 2, 1, 4, 5)).astype(BF)

    return dict(
        wq_t=tile_dh(wq).astype(np.float16),  # [N, 128, DCH, H]
        wk_t=tile_dh(wk).astype(np.float16),  # [KH, 128, DCH, H]
        wv_t=tile_dh(wv).astype(np.float16),
        wav_t=wav_t, wg_t=wg_t, wl_t=wl_t,
        q_scale=np.ascontiguousarray(
            (1.0 + np.asarray(inputs['q_norm_scale'], dtype=np.float32))
            .reshape(HCH, CH).T),                                 # [128, 2]
        k_scale=np.ascontiguousarray(
            (1.0 + np.asarray(inputs['k_norm_scale'], dtype=np.float32))
            .reshape(HCH, CH).T),
        post_attn=np.ascontiguousarray(
            (1.0 + np.asarray(inputs['post_attn_scale'], dtype=np.float32))
            .reshape(DCH, CH).T),                                 # [128, 16]
        post_ffw=np.ascontiguousarray(
            (1.0 + np.asarray(inputs['post_ffw_scale'], dtype=np.float32))
            .reshape(DCH, CH).T),
        skip=float(np.asarray(inputs['skip_scale']).reshape(-1)[0]),
    )


# ----------------------------------------------------------------------------
# device kernel builder
# ----------------------------------------------------------------------------

def _build_nc(W, OWN_OFF, EXTRA, skip):
    import concourse.bass as bass  # noqa: F401
    import concourse.tile as tile
    from concourse import mybir, bacc
    from contextlib import ExitStack

    F32 = mybir.dt.float32
    F32R = mybir.dt.float32r
    BF16 = mybir.dt.bfloat16
    F16 = mybir.dt.float16
    AF = mybir.ActivationFunctionType
    OP = mybir.AluOpType

    NSEG = W // SEG_CH
    OWN_SEG = OWN_OFF // SEG
    NSEG_X = EXTRA // SEG_CH if EXTRA else 0

    nc = bacc.Bacc()
    d_x = nc.declare_dram_parameter("x_t", [NSEG, CH, DCH, SEG], F16, isOutput=False)
    d_sc = nc.declare_dram_parameter("sc_t", [CH, 2, W * CH], F16, isOutput=False)
    d_bias = nc.declare_dram_parameter("bias_t", [NSEG, CH, SEG_CH, TQ], BF16, isOutput=False)
    d_wq = nc.declare_dram_parameter("wq_t", [N, CH, DCH, H], F16, isOutput=False)
    d_wk = nc.declare_dram_parameter("wk_t", [KH, CH, DCH, H], F16, isOutput=False)
    d_wv = nc.declare_dram_parameter("wv_t", [KH, CH, DCH, H], F16, isOutput=False)
    d_wav = nc.declare_dram_parameter("wav_t", [N, 4, CH, HCH, TQ], BF16, isOutput=False)
    d_wg = nc.declare_dram_parameter("wg_t", [2, F // CH, CH, DCH, CH], BF16, isOutput=False)
    d_wl = nc.declare_dram_parameter("wl_t", [FGROUPS, 4, CH, FPG, 4, CH], BF16, isOutput=False)
    d_qs = nc.declare_dram_parameter("q_scale", [CH, HCH], F32, isOutput=False)
    d_ks = nc.declare_dram_parameter("k_scale", [CH, HCH], F32, isOutput=False)
    d_pa = nc.declare_dram_parameter("post_attn", [CH, DCH], F32, isOutput=False)
    d_pf = nc.declare_dram_parameter("post_ffw", [CH, DCH], F32, isOutput=False)
    if EXTRA:
        d_kc = nc.declare_dram_parameter("kc_t", [KH, CH, HCH, EXTRA * CH], F16, isOutput=False)
        d_vc = nc.declare_dram_parameter("vc_t", [KH, NSEG_X, CH, SEG_CH, H], BF16, isOutput=False)
        d_biasc = nc.declare_dram_parameter("biasc_t", [NSEG_X, CH, SEG_CH, TQ], BF16, isOutput=False)
    d_out = nc.declare_dram_parameter("out_t", [CH, DCH, TQ], F16, isOutput=True)

    with tile.TileContext(nc) as tc, \
            nc.allow_low_precision(reason="bf16 weights / f32r activations"), \
            ExitStack() as ctx:
        cpool = ctx.enter_context(tc.tile_pool(name="const", bufs=1))
        ones_f = cpool.tile([CH, CH], F32)
        nc.vector.memset(ones_f[:], 1.0)
        ones = cpool.tile([CH, CH], F32R)
        nc.vector.tensor_copy(ones[:], ones_f[:])
        ones_b = cpool.tile([CH, 1], BF16)
        nc.vector.tensor_copy(ones_b[:], ones[:, :1])
        ones_brow = cpool.tile([1, CH], BF16)
        nc.vector.memset(ones_brow[:], 1.0)
        ones_h = cpool.tile([CH, 1], F16)
        nc.vector.tensor_copy(ones_h[:], ones[:, :1])
        eps1 = cpool.tile([1, 1], F32)
        nc.vector.memset(eps1[:], EPS)
        epsp = cpool.tile([CH, 1], F32)
        nc.vector.memset(epsp[:], EPS)
        qs_t = cpool.tile([CH, HCH], F32)
        nc.sync.dma_start(qs_t[:], d_qs[:])
        ks_t = cpool.tile([CH, HCH], F32)
        nc.sync.dma_start(ks_t[:], d_ks[:])
        pa_t = cpool.tile([CH, DCH], F32)
        nc.sync.dma_start(pa_t[:], d_pa[:])
        pf_t = cpool.tile([CH, DCH], F32)
        nc.sync.dma_start(pf_t[:], d_pf[:])
        sc_t = cpool.tile([CH, 2, W * CH], F16)
        nc.sync.dma_start(sc_t[:], d_sc[:])
        # ln(2^-40): compensates the 2^-40 pre-scale that keeps softmax sums
        # inside Ln's valid input range [-2^64, 2^64]
        ln240 = cpool.tile([1, 1], F32)
        nc.vector.memset(ln240[:], float(-40.0 * np.log(2.0)))

        ps_mm = ctx.enter_context(tc.tile_pool(name="ps_mm", bufs=6, space="PSUM"))
        ps_st = ctx.enter_context(tc.tile_pool(name="ps_st", bufs=2, space="PSUM"))
        cd = ctx.enter_context(tc.tile_pool(name="cdpool", bufs=1))

        def row_rstd(pool, stat_psum, inv_n, tag, ln_sink=None):
            """[1, n] PSUM sum-of-squares -> [1, n] f32r rstd in SBUF."""
            n = stat_psum.shape[-1]
            lnm = pool.tile([1, n], F32, tag=f"ln_{tag}", name="lnm")
            ln_i = nc.scalar.activation(out=lnm[:], in_=stat_psum[:], func=AF.Ln,
                                        bias=eps1[:], scale=inv_n)
            rstd = pool.tile([1, n], F32R, tag=f"rstd_{tag}", name="rstd")
            nc.scalar.activation(out=rstd[:], in_=lnm[:], func=AF.Exp, scale=-0.5)
            if ln_sink is not None:
                ln_sink.append(ln_i)
            return rstd

        def bcast(pool, rstd, tag, dtype=None):
            """[1, n] f32r -> [128, n] broadcast via PE + ACT copy."""
            n = rstd.shape[-1]
            bcp = ps_st.tile([CH, n], F32, tag="stat", name="bcp")
            nc.tensor.matmul(bcp[:], ones[:1, :], rstd[:], start=True, stop=True)
            bc = pool.tile([CH, n], dtype or F32, tag=f"bc_{tag}", name="bc")
            nc.scalar.activation(out=bc[:], in_=bcp[:], func=AF.Copy, scale=1.0)
            return bc

        # ==================================================================
        # attention
        # ==================================================================
        with tc.tile_pool(name="bpool", bufs=1) as bpool:
            qall = bpool.tile([CH, N, HCH, TQ], F16)      # 16KB/part
            Oall = bpool.tile([CH, N, HCH, TQ], BF16)      # 16KB/part
            sums = bpool.tile([1, N, TQ], BF16)

            # ---- KV pools open early so segment-0 x/bias/weights prefetch
            # ahead of the Q-phase DMA stream ----
            with tc.tile_pool(name="xs_s", bufs=2) as xss, \
                    tc.tile_pool(name="bias_s", bufs=2) as bss:
                xs0 = xss.tile([CH, DCH, SEG], F16, tag="xs", name="xs")
                # split into per-quarter DMAs so the first Q matmuls start
                # as soon as the first dc chunks land
                for q4 in range(4):
                    nc.scalar.dma_start(xs0[:, 4 * q4:4 * q4 + 4, :],
                                        d_x[OWN_SEG][:, 4 * q4:4 * q4 + 4, :])
                bias0 = bss.tile([CH, SEG_CH, TQ], BF16, tag="bias", name="bias_seg")
                nc.scalar.dma_start(bias0[:], d_bias[OWN_SEG])
                xq = xs0
                cos_o = sc_t[:, 0, OWN_OFF:OWN_OFF + TQ]
                sin_o = sc_t[:, 1, OWN_OFF:OWN_OFF + TQ]
                with tc.tile_pool(name="qph1", bufs=1) as qph1, \
                        tc.tile_pool(name="qph2", bufs=2) as qph2, \
                        tc.tile_pool(name="wq_s", bufs=3) as wqs:
                    qraw = qph1.tile([CH, N, HCH, TQ], F16, tag="qraw")
                    qrow = qph1.tile([1, N * TQ], F32R, tag="qrow")
                    QB = N // 2
                    for qb in range(2):
                        for n in range(qb * QB, (qb + 1) * QB):
                            wq_t = wqs.tile([CH, DCH, H], F16, tag="wq", name="wq_t")
                            nc.sync.dma_start(wq_t[:, :, 0:CH], d_wq[n][:, :, 0:CH])
                            nc.sync.dma_start(wq_t[:, :, CH:H], d_wq[n][:, :, CH:H])
                            for hc in range(HCH):
                                qp = ps_mm.tile([CH, TQ], F32, tag="mm", name="qp")
                                for dc in range(DCH):
                                    nc.tensor.matmul(qp[:],
                                                     wq_t[:, dc, hc * CH:(hc + 1) * CH],
                                                     xq[:, dc, :],
                                                     start=(dc == 0), stop=(dc == DCH - 1))
                                sq = qph2.tile([CH, TQ], F16, tag="sq", name="sq")
                                nc.scalar.activation(out=sq[:], in_=qp[:], func=AF.Square, scale=1.0)
                                qst = ps_st.tile([1, TQ], F32, tag="stat", name="qst")
                                nc.tensor.matmul(qst[:], ones_h[:, :1], sq[:],
                                                 start=True, stop=True)
                                nc.vector.tensor_scalar_mul(qraw[:, n, hc, :], qp[:],
                                                            qs_t[:, hc:hc + 1])
                                if hc == 0:
                                    nc.scalar.activation(out=qrow[:, n * TQ:(n + 1) * TQ],
                                                         in_=qst[:], func=AF.Copy, scale=1.0)
                                else:
                                    nc.vector.tensor_tensor(qrow[:, n * TQ:(n + 1) * TQ],
                                                            qrow[:, n * TQ:(n + 1) * TQ],
                                                            qst[:], OP.add)
                        qsl = qrow[:, qb * QB * TQ:(qb + 1) * QB * TQ]
                        nc.scalar.activation(out=qsl, in_=qsl, func=AF.Ln,
                                             bias=eps1[:], scale=1.0 / H)
                        nc.scalar.activation(out=qsl, in_=qsl, func=AF.Exp, scale=-0.5)
                        for n in range(qb * QB, (qb + 1) * QB):
                            bc = bcast(qph2, qrow[:, n * TQ:(n + 1) * TQ], "q", dtype=F16)
                            for hc in range(HCH):
                                nc.vector.tensor_tensor(qraw[:, n, hc, :], qraw[:, n, hc, :],
                                                        bc[:], OP.mult)
                            t0_ = qph2.tile([CH, TQ], F16, tag="qt0", name="t0_")
                            t1_ = qph2.tile([CH, TQ], F16, tag="qt1", name="t1_")
                            nc.vector.tensor_tensor(t0_[:], qraw[:, n, 0, :], sin_o, OP.mult)
                            nc.vector.tensor_tensor(t1_[:], qraw[:, n, 1, :], sin_o, OP.mult)
                            nc.vector.tensor_tensor(qraw[:, n, 0, :], qraw[:, n, 0, :], cos_o, OP.mult)
                            nc.vector.tensor_tensor(qraw[:, n, 1, :], qraw[:, n, 1, :], cos_o, OP.mult)
                            nc.vector.tensor_tensor(qall[:, n, 0, :], qraw[:, n, 0, :], t1_[:], OP.subtract)
                            nc.vector.tensor_tensor(qall[:, n, 1, :], qraw[:, n, 1, :], t0_[:], OP.add)


                # ---- KV + attend, software-pipelined across segments:
                # the K/V projection + stats + normalize/rope of segment s+1
                # is emitted BEFORE the attends of segment s, so the PE's
                # attend matmuls (which wait on the tanh/exp chain) always
                # have the next segment's projection matmuls to overlap with,
                # and vice versa. ----
                kvctx = ExitStack()
                ap1 = kvctx.enter_context(tc.tile_pool(name="ap1", bufs=2))
                ap2 = kvctx.enter_context(tc.tile_pool(name="ap2", bufs=2))
                krw = kvctx.enter_context(tc.tile_pool(name="krw", bufs=2))
                ktp = kvctx.enter_context(tc.tile_pool(name="ktp", bufs=8))
                vtp = kvctx.enter_context(tc.tile_pool(name="vtp", bufs=8))
                epp = kvctx.enter_context(tc.tile_pool(name="epp", bufs=3))
                wks = kvctx.enter_context(tc.tile_pool(name="wk_s", bufs=2))

                def seg_ranges(seg):
                    """Per key-chunk query ranges implied by the sliding
                    window + causality (chunk-granular; the 0/1 bias handles
                    the exact interior masking). Ordered so the first range
                    covers the union (its psum `start` zeroes everything the
                    later sub-range matmuls accumulate into)."""
                    rngs = []
                    for st in range(SEG_CH):
                        o0 = seg * SEG + st * CH - OWN_OFF
                        q_lo = max(0, o0)
                        q_hi = min(TQ, o0 + CH - 1 + WINDOW)
                        if q_lo < q_hi:
                            rngs.append((st, q_lo, q_hi))
                    rngs.sort(key=lambda r: r[1] - r[2])
                    assert rngs and all(r[1] >= rngs[0][1] and r[2] <= rngs[0][2]
                                        for r in rngs)
                    return rngs

                FULL_RANGES = [(st, 0, TQ) for st in range(SEG_CH)]

                def attend_seg(kts, vts, bias_seg, first, ranges=None, post=None,
                               mid=None):
                    """Attend all N heads of one segment, with a one-head
                    lookahead: the logits/tanh/exp of head n are emitted
                    before the AV matmuls of head n-1, hiding the softcap
                    chain latency behind the next head's PE work."""
                    ranges = ranges or FULL_RANGES
                    _, u0, u1 = ranges[0]

                    def flush_head(n_head, eps_):
                        v_seg = vts[n_head // G]
                        for hc in range(HCH):
                            op = ps_mm.tile([CH, TQ], F32, tag="mm", name="op")
                            for i, (st, q0, q1) in enumerate(ranges):
                                nc.tensor.matmul(
                                    op[:, q0:q1], v_seg[:, st, hc * CH:(hc + 1) * CH],
                                    eps_[i][:, :q1 - q0],
                                    start=(i == 0), stop=(i == len(ranges) - 1),
                                    skip_group_check=True)
                            if first:
                                nc.scalar.activation(out=Oall[:, n_head, hc, :],
                                                     in_=op[:], func=AF.Copy, scale=1.0)
                            else:
                                nc.vector.tensor_tensor(Oall[:, n_head, hc, u0:u1],
                                                        Oall[:, n_head, hc, u0:u1],
                                                        op[:, u0:u1], OP.add)
                        sp = ps_st.tile([1, TQ], F32, tag="stat", name="sp")
                        for i, (st, q0, q1) in enumerate(ranges):
                            nc.tensor.matmul(sp[:, q0:q1], ones_b[:, :1],
                                             eps_[i][:, :q1 - q0],
                                             start=(i == 0), stop=(i == len(ranges) - 1),
                                             skip_group_check=True)
                        if first:
                            nc.scalar.activation(out=sums[:, n_head, :], in_=sp[:],
                                                 func=AF.Copy, scale=1.0)
                        else:
                            nc.vector.tensor_tensor(sums[:, n_head, u0:u1],
                                                    sums[:, n_head, u0:u1],
                                                    sp[:, u0:u1], OP.add)
                        if post is not None:
                            post(n_head)
                        if mid is not None:
                            mid(n_head)

                    prev = None
                    for n_head in range(N):
                        kT_seg = kts[n_head // G]
                        eps_ = []
                        for st, q0, q1 in ranges:
                            nq = q1 - q0
                            lg = ps_mm.tile([CH, TQ], F32, tag="mm", name="lg")
                            for hc in range(HCH):
                                nc.tensor.matmul(
                                    lg[:, :nq], kT_seg[:, hc, st * CH:(st + 1) * CH],
                                    qall[:, n_head, hc, q0:q1],
                                    start=(hc == 0), stop=(hc == HCH - 1))
                            th = epp.tile([CH, TQ], F32R, tag="tanh", name="th", bufs=2)
                            nc.scalar.activation(out=th[:, :nq], in_=lg[:, :nq],
                                                 func=AF.Tanh, scale=1.0 / SOFT_CAP)
                            ep = epp.tile([CH, TQ], BF16, tag="expp", name="ep", bufs=8)
                            nc.scalar.activation(out=ep[:, :nq], in_=th[:, :nq],
                                                 func=AF.Exp, scale=SOFT_CAP)
                            nc.vector.tensor_tensor(ep[:, :nq], ep[:, :nq],
                                                    bias_seg[:, st, q0:q1], OP.mult)
                            eps_.append(ep)
                        if prev is not None:
                            flush_head(*prev)
                        prev = (n_head, eps_)
                    flush_head(*prev)

                def project_segment(seg, dma0):
                    """K/V projection + stats + normalize + rope for one
                    segment; returns (kts, vts, bias_seg) ready to attend."""
                    ssl = slice(seg * SEG, (seg + 1) * SEG)
                    if dma0 is not None:
                        xs, bias_seg = dma0
                    else:
                        xs = xss.tile([CH, DCH, SEG], F16, tag="xs", name="xs")
                        nc.scalar.dma_start(xs[:], d_x[seg])
                        bias_seg = bss.tile([CH, SEG_CH, TQ], BF16, tag="bias", name="bias_seg")
                        nc.scalar.dma_start(bias_seg[:], d_bias[seg])
                    cos_s = sc_t[:, 0, ssl]
                    sin_s = sc_t[:, 1, ssl]

                    kts, vts = [], []
                    krow = krw.tile([1, KH * SEG], F32R, tag="krow", name="krow")
                    vst = krw.tile([CH, KH, SEG_CH], F32, tag="vst", name="vst")
                    for kh in range(KH):
                        wk_t = wks.tile([CH, DCH, H], F16, tag="wkv", name="wk_t")
                        nc.scalar.dma_start(wk_t[:], d_wk[kh])
                        kT_seg = ktp.tile([CH, HCH, SEG], F16, tag="kt", name="kT_seg")
                        for hc in range(HCH):
                            kp = ps_mm.tile([CH, SEG], F32, tag="mm", name="kp")
                            for dc in range(DCH):
                                nc.tensor.matmul(kp[:],
                                                 wk_t[:, dc, hc * CH:(hc + 1) * CH],
                                                 xs[:, dc, :],
                                                 start=(dc == 0), stop=(dc == DCH - 1))
                            ksq = ap1.tile([CH, SEG], F16, tag="sq", name="ksq")
                            nc.scalar.activation(out=ksq[:], in_=kp[:], func=AF.Square, scale=1.0)
                            kst = ps_st.tile([1, SEG], F32, tag="stat", name="kst")
                            nc.tensor.matmul(kst[:], ones_h[:, :1], ksq[:],
                                             start=True, stop=True)
                            nc.vector.tensor_scalar_mul(kT_seg[:, hc, :], kp[:],
                                                        ks_t[:, hc:hc + 1])
                            if hc == 0:
                                nc.scalar.activation(out=krow[:, kh * SEG:(kh + 1) * SEG],
                                                     in_=kst[:], func=AF.Copy, scale=1.0)
                            else:
                                nc.vector.tensor_tensor(krow[:, kh * SEG:(kh + 1) * SEG],
                                                        krow[:, kh * SEG:(kh + 1) * SEG],
                                                        kst[:], OP.add)
                        kts.append(kT_seg)

                        wv_t = wks.tile([CH, DCH, H], F16, tag="wkv", name="wv_t")
                        nc.scalar.dma_start(wv_t[:], d_wv[kh])
                        v_seg = vtp.tile([CH, SEG_CH, H], BF16, tag="vt", name="v_seg")
                        for sp2 in range(2):
                            vp = ps_mm.tile([CH, 2, H], F32, tag="mm", name="vp")
                            for sl in range(2):
                                st = sp2 * 2 + sl
                                for dc in range(DCH):
                                    nc.tensor.matmul(vp[:, sl, :],
                                                     xs[:, dc, st * CH:(st + 1) * CH],
                                                     wv_t[:, dc, :],
                                                     start=(dc == 0), stop=(dc == DCH - 1))
                            vsq = ap1.tile([CH, 2, H], F16, tag="sq", name="vsq")
                            nc.scalar.activation(out=vsq[:], in_=vp[:], func=AF.Square, scale=1.0)
                            for sl in range(2):
                                st = sp2 * 2 + sl
                                nc.vector.reduce_sum(vst[:, kh, st:st + 1], vsq[:, sl, :],
                                                     axis=mybir.AxisListType.X)
                            nc.scalar.activation(out=v_seg[:, sp2 * 2:(sp2 + 1) * 2, :],
                                                 in_=vp[:], func=AF.Copy, scale=1.0)
                        vts.append(v_seg)

                    # -- one Ln/Exp pair per segment for all k and v stats --
                    nc.scalar.activation(out=krow[:], in_=krow[:], func=AF.Ln,
                                         bias=eps1[:], scale=1.0 / H)
                    nc.scalar.activation(out=vst[:], in_=vst[:], func=AF.Ln,
                                         bias=epsp[:], scale=1.0 / H)
                    nc.scalar.activation(out=krow[:], in_=krow[:], func=AF.Exp, scale=-0.5)
                    krstd = krow
                    nc.scalar.activation(out=vst[:], in_=vst[:], func=AF.Exp, scale=-0.5)
                    vrstd = vst

                    return kts, vts, bias_seg, krstd, vrstd, cos_s, sin_s

                def project_phase2(ph1, khs=range(KH)):
                    # normalize + rope (DVE + bck broadcasts on the PE, gated
                    # on the Ln/Exp chain -- emitted AFTER the previous
                    # segment's attends so those never wait on it)
                    kts, vts, _, krstd, vrstd, cos_s, sin_s = ph1
                    for kh in khs:
                        kT_seg, v_seg = kts[kh], vts[kh]
                        bck = bcast(ap2, krstd[:, kh * SEG:(kh + 1) * SEG], "k", dtype=F16)
                        for hc in range(HCH):
                            nc.vector.tensor_tensor(kT_seg[:, hc, :], kT_seg[:, hc, :],
                                                    bck[:], OP.mult)
                        t0_ = ap2.tile([CH, SEG], F16, tag="kt0", name="t0_")
                        t1_ = ap2.tile([CH, SEG], F16, tag="kt1", name="t1_")
                        nc.vector.tensor_tensor(t0_[:], kT_seg[:, 0, :], sin_s, OP.mult)
                        nc.vector.tensor_tensor(t1_[:], kT_seg[:, 1, :], sin_s, OP.mult)
                        nc.vector.tensor_tensor(kT_seg[:, 0, :], kT_seg[:, 0, :], cos_s, OP.mult)
                        nc.vector.tensor_tensor(kT_seg[:, 1, :], kT_seg[:, 1, :], cos_s, OP.mult)
                        nc.vector.tensor_tensor(kT_seg[:, 0, :], kT_seg[:, 0, :], t1_[:], OP.subtract)
                        nc.vector.tensor_tensor(kT_seg[:, 1, :], kT_seg[:, 1, :], t0_[:], OP.add)
                        for st in range(SEG_CH):
                            nc.vector.tensor_scalar_mul(v_seg[:, st, :], v_seg[:, st, :],
                                                        vrstd[:, kh, st:st + 1])

                def normalize_head(n_head):
                    # divide Oall by the softmax sum: PE broadcast + fast
                    # DVE reciprocal, emitted per head right after its last
                    # attend so attn_vec can start while later heads attend
                    bcp = ps_st.tile([CH, TQ], F32, tag="stat", name="bcp")
                    nc.tensor.matmul(bcp[:], ones_brow[:], sums[:, n_head, :],
                                     start=True, stop=True)
                    bc = ap2.tile([CH, TQ], F32, tag="bc_s", name="bc", bufs=1)
                    nc.vector.reciprocal_approx_fast(bc[:], bcp[:])
                    for hc in range(HCH):
                        nc.vector.tensor_tensor(Oall[:, n_head, hc, :],
                                                Oall[:, n_head, hc, :], bc[:], OP.mult)

                seg_order = [OWN_SEG] + [s for s in range(NSEG) if s != OWN_SEG]
                pend = None
                for idx, seg in enumerate(seg_order):
                    dma0 = (xs0, bias0) if idx == 0 else None
                    ph1 = project_segment(seg, dma0)
                    if pend is not None:
                        # normalize kv-head 0 first; the remaining kv-heads'
                        # normalizes are interleaved into the previous
                        # segment's attend stream (after heads 1/3/5) so the
                        # attends never outrun the normalize production
                        project_phase2(ph1, range(0, 1))

                        def mid(n_head, _ph1=ph1):
                            if n_head in (1, 3, 5):
                                project_phase2(_ph1, [n_head // 2 + 1])

                        attend_seg(*pend, mid=mid)
                    else:
                        project_phase2(ph1)
                    pend = (ph1[0], ph1[1], ph1[2], idx == 0, seg_ranges(seg),
                            None)
                kts_p, vts_p, bias_p, first_p, rng_p, _ = pend
                attend_seg(kts_p, vts_p, bias_p, first_p, rng_p,
                           post=None if NSEG_X else normalize_head)

                for sx in range(NSEG_X):
                    bias_seg = bss.tile([CH, SEG_CH, TQ], BF16, tag="bias", name="bias_seg")
                    nc.sync.dma_start(bias_seg[:], d_biasc[sx])
                    kts_x, vts_x = [], []
                    for kh in range(KH):
                        kT_seg = ktp.tile([CH, HCH, SEG], F16, tag="kt", name="kT_seg")
                        nc.sync.dma_start(kT_seg[:],
                                          d_kc[kh][:, :, sx * SEG:(sx + 1) * SEG])
                        v_seg = vtp.tile([CH, SEG_CH, H], BF16, tag="vt", name="v_seg")
                        nc.sync.dma_start(v_seg[:], d_vc[kh, sx])
                        kts_x.append(kT_seg)
                        vts_x.append(v_seg)
                    attend_seg(kts_x, vts_x, bias_seg, False, None,
                               post=normalize_head if sx == NSEG_X - 1 else None)

                kvctx.close()

            # ---- normalize O (divide by softmax sum), then attn_vec ----
            with tc.tile_pool(name="avacc", bufs=1) as avaccp:
              with tc.tile_pool(name="avp", bufs=3) as avpool, \
                    tc.tile_pool(name="wav_s", bufs=8) as wavs:
                attn_acc = avaccp.tile([CH, DCH, TQ], F32)
                pa_stat = ps_st.tile([1, TQ], F32, tag="stat", name="pa_stat")
                for dcq in range(4):
                    wav_ts = []
                    for n_head in range(N):
                        wav_t = wavs.tile([CH, HCH, TQ], BF16, tag="wavf", name="wav_t")
                        nc.sync.dma_start(wav_t[:], d_wav[n_head, dcq])
                        wav_ts.append(wav_t)
                    for dcl in range(4):
                        dc = dcq * 4 + dcl
                        # two half-groups (heads 0-3 / 4-7): the first half's
                        # matmuls start as soon as 4 heads are normalized
                        avp_a = ps_mm.tile([CH, TQ], F32, tag="mm", name="avp_a")
                        i = 0
                        for n_head in range(N // 2):
                            for hc in range(HCH):
                                nc.tensor.matmul(avp_a[:],
                                                 wav_ts[n_head][:, hc, dcl * CH:(dcl + 1) * CH],
                                                 Oall[:, n_head, hc, :],
                                                 start=(i == 0), stop=(i == N - 1))
                                i += 1
                        avp_b = ps_mm.tile([CH, TQ], F32, tag="mm", name="avp_b")
                        i = 0
                        for n_head in range(N // 2, N):
                            for hc in range(HCH):
                                nc.tensor.matmul(avp_b[:],
                                                 wav_ts[n_head][:, hc, dcl * CH:(dcl + 1) * CH],
                                                 Oall[:, n_head, hc, :],
                                                 start=(i == 0), stop=(i == N - 1))
                                i += 1
                        nc.scalar.activation(out=attn_acc[:, dc, :], in_=avp_a[:],
                                             func=AF.Copy, scale=1.0)
                        nc.vector.tensor_tensor(attn_acc[:, dc, :], attn_acc[:, dc, :],
                                                avp_b[:], OP.add)
                        sqa = avpool.tile([CH, TQ], F32R, tag="sqa", name="sqa")
                        nc.scalar.activation(out=sqa[:], in_=attn_acc[:, dc, :],
                                             func=AF.Square, scale=1.0)
                        nc.tensor.matmul(pa_stat[:], ones[:, :1], sqa[:],
                                         start=(dc == 0), stop=(dc == DCH - 1))

              # ---- post-attn norm + residual; pre-ffw norm ----
              if True:
                attn_out = cd.tile([CH, DCH, TQ], F16)
                ffw_in = cd.tile([CH, DCH, TQ], BF16)
                with tc.tile_pool(name="phc", bufs=1) as pc1, \
                        tc.tile_pool(name="phc2", bufs=2) as pc2:
                    x_own = pc1.tile([CH, DCH, SEG], F16, tag="xown")
                    nc.sync.dma_start(x_own[:], d_x[OWN_SEG])
                    rstd_pa = row_rstd(pc2, pa_stat, 1.0 / D, "pa")
                    bc = bcast(pc2, rstd_pa, "pa")
                    pf_stat = ps_st.tile([1, TQ], F32, tag="stat", name="pf_stat")
                    for dc in range(DCH):
                        tt = pc2.tile([CH, TQ], F32, tag="catmp", name="tt")
                        nc.vector.scalar_tensor_tensor(tt[:], attn_acc[:, dc, :],
                                                       pa_t[:, dc:dc + 1], bc[:],
                                                       OP.mult, OP.mult)
                        nc.vector.scalar_tensor_tensor(attn_out[:, dc, :],
                                                       x_own[:, dc, :], float(skip),
                                                       tt[:], OP.mult, OP.add)
                        sqf = pc2.tile([CH, TQ], F32R, tag="sqf", name="sqf")
                        nc.scalar.activation(out=sqf[:], in_=attn_out[:, dc, :],
                                             func=AF.Square, scale=1.0)
                        nc.tensor.matmul(pf_stat[:], ones[:, :1], sqf[:],
                                         start=(dc == 0), stop=(dc == DCH - 1))
                    rstd_pf = row_rstd(pc2, pf_stat, 1.0 / D, "pf")
                    bc2 = bcast(pc2, rstd_pf, "pf")
                    for dc in range(DCH):
                        nc.vector.tensor_tensor(ffw_in[:, dc, :], attn_out[:, dc, :], bc2[:], OP.mult)


        # ==================================================================
        # FFN
        # ==================================================================
        if True:
            with tc.tile_pool(name="dp1", bufs=1) as dp1, \
                    tc.tile_pool(name="dp2", bufs=2) as dp2, \
                    tc.tile_pool(name="actp", bufs=FHALF) as actp, \
                    tc.tile_pool(name="wg_s", bufs=4) as wgs, \
                    tc.tile_pool(name="wl_s", bufs=4) as wls:
                ffw_acc = dp1.tile([CH, DCH, TQ], F32)
                of_stat = None
                for half in range(2):
                    acts = []
                    for fgl in range(FHALF):
                        fg = half * FHALF + fgl
                        act = actp.tile([CH, FPG, TQ], BF16, tag="act", name=f"act{fgl}")
                        fc_start = 0
                        if half == 0 and fgl == 0:
                            # dc-major across the first two f-chunks' four
                            # psum groups: the PE consumes ffw_in chunk by
                            # chunk as the post-attn chain produces it,
                            # instead of head-of-line blocking on chunk 15
                            fc_start = 2
                            wgts, pgs = [], []
                            for fc in range(fc_start):
                                for g01 in range(2):
                                    wg_t = wgs.tile([CH, DCH, CH], BF16, tag="wg", name="wg_t")
                                    nc.sync.dma_start(wg_t[:], d_wg[g01, fg * FPG + fc])
                                    wgts.append(wg_t)
                                    pgs.append(ps_mm.tile([CH, TQ], F32, tag="mm",
                                                          name=f"pgi{fc}{g01}"))
                            for dc in range(DCH):
                                for i in range(2 * fc_start):
                                    nc.tensor.matmul(pgs[i][:], wgts[i][:, dc, :],
                                                     ffw_in[:, dc, :],
                                                     start=(dc == 0), stop=(dc == DCH - 1))
                            for fc in range(fc_start):
                                gel = dp2.tile([CH, TQ], F32, tag="gel", name="gel")
                                nc.scalar.activation(out=gel[:], in_=pgs[2 * fc][:],
                                                     func=AF.Gelu_apprx_tanh, scale=1.0)
                                nc.vector.tensor_tensor(act[:, fc, :], gel[:],
                                                        pgs[2 * fc + 1][:], OP.mult)
                        for fc in range(fc_start, FPG):
                            fglob = fg * FPG + fc
                            gp = []
                            for g01 in range(2):
                                wg_t = wgs.tile([CH, DCH, CH], BF16, tag="wg", name="wg_t")
                                nc.sync.dma_start(wg_t[:], d_wg[g01, fglob])
                                pg = ps_mm.tile([CH, TQ], F32, tag="mm", name=f"pg{g01}")
                                for dc in range(DCH):
                                    nc.tensor.matmul(pg[:], wg_t[:, dc, :], ffw_in[:, dc, :],
                                                     start=(dc == 0), stop=(dc == DCH - 1))
                                gp.append(pg)
                            gel = dp2.tile([CH, TQ], F32, tag="gel", name="gel")
                            nc.scalar.activation(out=gel[:], in_=gp[0][:],
                                                 func=AF.Gelu_apprx_tanh, scale=1.0)
                            nc.vector.tensor_tensor(act[:, fc, :], gel[:], gp[1][:], OP.mult)
                        acts.append(act)
                    for dcq in range(4):
                        wl_ts = []
                        for fgl in range(FHALF):
                            fg = half * FHALF + fgl
                            wl_t = wls.tile([CH, FPG, 4, CH], BF16, tag="wl", name="wl_t")
                            nc.sync.dma_start(wl_t[:], d_wl[fg, dcq])
                            wl_ts.append(wl_t)
                        for dcl in range(4):
                            dc = dcq * 4 + dcl
                            pf = ps_mm.tile([CH, TQ], F32, tag="mm", name="pf")
                            i = 0
                            for fgl in range(FHALF):
                                for fc in range(FPG):
                                    nc.tensor.matmul(pf[:], wl_ts[fgl][:, fc, dcl, :],
                                                     acts[fgl][:, fc, :],
                                                     start=(i == 0), stop=(i == FHALF * FPG - 1))
                                    i += 1
                            if half == 0:
                                nc.scalar.activation(out=ffw_acc[:, dc, :], in_=pf[:],
                                                     func=AF.Copy, scale=1.0)
                            else:
                                nc.vector.tensor_tensor(ffw_acc[:, dc, :], ffw_acc[:, dc, :],
                                                        pf[:], OP.add)
                                sqo = dp2.tile([CH, TQ], F32R, tag="sqo", name="sqo")
                                nc.scalar.activation(out=sqo[:], in_=ffw_acc[:, dc, :],
                                                     func=AF.Square, scale=1.0)
                                if of_stat is None:
                                    of_stat = ps_st.tile([1, TQ], F32, tag="stat", name="of_stat")
                                nc.tensor.matmul(of_stat[:], ones[:, :1], sqo[:],
                                                 start=(dc == 0), stop=(dc == DCH - 1))
                # post-ffw norm + final residual (into attn_out, then store)
                rstd_of = row_rstd(dp2, of_stat, 1.0 / D, "of")
                bc3 = bcast(dp2, rstd_of, "of")
                for dc in range(DCH):
                    tt = dp2.tile([CH, TQ], F32, tag="fftmp", name="tt")
                    nc.vector.scalar_tensor_tensor(tt[:], ffw_acc[:, dc, :],
                                                   pf_t[:, dc:dc + 1], bc3[:],
                                                   OP.mult, OP.mult)
                    nc.vector.tensor_tensor(attn_out[:, dc, :], attn_out[:, dc, :], tt[:], OP.add)
                nc.sync.dma_start(d_out[:], attn_out[:])

    nc.finalize()
    return nc


_NC_CACHE = {}


def _in_maps(plan, w):
    in_maps = []
    for c in range(N_CORES):
        e = plan['per_core'][c]
        m = dict(x_t=e['x_t'], sc_t=e['sc_t'], bias_t=e['bias_t'],
                 wq_t=w['wq_t'], wk_t=w['wk_t'], wv_t=w['wv_t'],
                 wav_t=w['wav_t'], wg_t=w['wg_t'], wl_t=w['wl_t'],
                 q_scale=w['q_scale'], k_scale=w['k_scale'],
                 post_attn=w['post_attn'], post_ffw=w['post_ffw'])
        if plan['EXTRA']:
            m.update(kc_t=e['kc_t'], vc_t=e['vc_t'], biasc_t=e['biasc_t'])
        in_maps.append(m)
    return in_maps


def _nc_key(plan, w):
    return (plan['W'], plan['OWN_OFF'], plan['EXTRA'], w['skip'])


def kernel(**inputs) -> np.ndarray:
    from concourse.bass_utils import run_bass_kernel_spmd

    plan = _plan(inputs)
    w = _prep_weights(inputs)
    key = _nc_key(plan, w)
    if key not in _NC_CACHE:
        _NC_CACHE[key] = _build_nc(*key)
    nc = _NC_CACHE[key]

    res = run_bass_kernel_spmd(nc, _in_maps(plan, w), core_ids=list(range(N_CORES)))

    out = np.zeros((B, T, D), np.float32)
    for c in range(N_CORES):
        e = plan['per_core'][c]
        o = res.results[c]['out_t']          # [128, DCH, TQ]
        out[e['b'], e['t0']:e['t0'] + TQ] = o.transpose(2, 1, 0).reshape(TQ, D)
    return out



# revision 47
# speedup vs baseline: 1.0054x; 1.0054x over previous
"""Trainium2 Bass kernel for nn_Block_16441134809284 (sparse_attention block).

Self-contained: token-parallel over 8 NeuronCores (2 batches x 4 slices of 512
tokens). Each core computes its 512 output tokens end-to-end; KV for the
sliding window is recomputed per core from a zero-padded token window, so no
collectives are needed.

v3 design notes (on top of v2):
- Software-pipelined segment schedule: K/V projection matmuls + stats of
  segment s+1 are emitted before the attends of segment s, and the
  normalize/rope (which gates on the per-segment stats Ln/Exp) is emitted
  after them, so the PE never queues behind the rstd chain.
- attend processes all 8 heads with a one-head lookahead (logits/softcap of
  head n before the AV matmuls of head n-1), and the final segment's
  attends interleave the per-head 1/softmax-sum normalize (PE broadcast +
  fast-approx DVE reciprocal) so attn_vec starts while later heads attend.
- Chunk-granular triangular skip: per key-chunk query ranges implied by the
  sliding window + causality cut ~25% of the logits/softcap/AV work; the
  first processed range covers the union so the PSUM `start` zeroing stays
  valid.
- The pre-attention rms_norm(x) is absorbed by the q/k/v rms_norms (all three
  normalize per token after the projection, and rms_norm is scale-invariant),
  so q/k/v are projected directly from x with (1+pre_attn_scale) folded into
  the weights.
- All large tensors ship as bf16, pre-tiled partition-major so every DMA is
  128 contiguous per-partition chunks (large descriptors). Activations stay
  fp32 (float32r) except where noted; matmuls mix bf16 weights (stationary)
  with f32r moving operands, which the PE allows.
- rstd = exp(-0.5*ln(mean+eps)) on ScalarE: avoids the banned Rsqrt, the slow
  DVE reciprocal, and keeps the attention phase within two ACT table sets
  (exp_and_others / natural_log_exp_and_others).
- Softmax without max-subtraction (logits are soft-capped to +-50); masking is
  a 0/1 multiplier on exp values.
"""
import sys

for _p in ("/opt/trn_rl_repo", "/root/.axon_site/_ro/trn_rl_repo"):
    if _p not in sys.path:
        sys.path.insert(0, _p)

import numpy as np
import ml_dtypes

BF = ml_dtypes.bfloat16

K_MASK = -2.3819763e+38
SOFT_CAP = 50.0
WINDOW = 1024
ROPE_BASE = 10000.0

B, T, D, N, KH, H, F = 2, 2048, 2048, 8, 4, 256, 8192
CACHE = 4096
N_CORES = 8
SLICES = N_CORES // B
TQ = T // SLICES          # 512
EPS = 1e-6
CH = 128                  # chunk (partition) size
DCH = D // CH             # 16
HCH = H // CH             # 2
SEG_CH = 4                # kv chunks per attention segment (512 tokens)
SEG = SEG_CH * CH         # 512
FPG = 8                   # F-chunks per FFN group
FGROUPS = F // CH // FPG  # 8
FHALF = FGROUPS // 2      # 4
G = N // KH               # 2 query heads per kv head


# ----------------------------------------------------------------------------
# host-side planning
# ----------------------------------------------------------------------------

def _plan(inputs):
    attn_mask = np.asarray(inputs['attn_mask'])
    seg_pos = np.asarray(inputs['segment_pos']).astype(np.int64)
    cache_pos_in = np.asarray(inputs['cache_positions']).astype(np.int64)
    end_index = np.asarray(inputs['end_index']).astype(np.int64)
    x = np.asarray(inputs['x'], dtype=np.float32)

    slot_of_tok = (end_index[:, None] + np.arange(T)[None, :]) % CACHE
    old_slots = (end_index[:, None] + T + np.arange(CACHE - T)[None, :]) % CACHE

    cache_pos = cache_pos_in.copy()
    bidx = np.arange(B)[:, None]
    cache_pos[bidx, slot_of_tok] = seg_pos
    sliding = (cache_pos[:, None, :] > seg_pos[:, :, None] - WINDOW) & \
              (cache_pos[:, None, :] < seg_pos[:, :, None] + WINDOW)
    mask = attn_mask & sliding                      # [B, T(query), S(slot)]

    mask_tok = np.take_along_axis(mask, slot_of_tok[:, None, :], axis=2)
    mask_old = np.take_along_axis(mask, old_slots[:, None, :], axis=2)

    L_need = R_need = 0
    cache_chunks_needed = 0
    n_old = CACHE - T
    for b in range(B):
        for s in range(SLICES):
            t0 = s * TQ
            used = mask_tok[b, t0:t0 + TQ].any(axis=0)
            cidx = np.nonzero(used.reshape(T // CH, CH).any(axis=1))[0]
            if len(cidx):
                L_need = max(L_need, t0 // CH - int(cidx[0]))
                R_need = max(R_need, int(cidx[-1]) + 1 - (t0 + TQ) // CH)
            used_o = mask_old[b, t0:t0 + TQ].any(axis=0)
            co = np.nonzero(used_o.reshape(n_old // CH, CH).any(axis=1))[0]
            cache_chunks_needed = max(cache_chunks_needed, len(co))

    L_need = ((max(L_need, 0) + SEG_CH - 1) // SEG_CH) * SEG_CH
    R_need = max(R_need, 0)
    W = L_need + TQ // CH + R_need
    W = ((W + SEG_CH - 1) // SEG_CH) * SEG_CH
    OWN_OFF = L_need * CH
    KV_TOK = W * CH
    NSEG = W // SEG_CH
    EXTRA = ((cache_chunks_needed + SEG_CH - 1) // SEG_CH) * SEG_CH \
        if cache_chunks_needed else 0

    per_core = []
    frac = 2.0 * np.arange(H // 2, dtype=np.float32) / np.float32(H)
    timescale = np.float32(ROPE_BASE) ** frac
    for c in range(N_CORES):
        b, s = divmod(c, SLICES)
        t0 = s * TQ
        toks = np.arange(t0 - OWN_OFF, t0 - OWN_OFF + KV_TOK)
        valid = (toks >= 0) & (toks < T)
        tv = np.clip(toks, 0, T - 1)

        xw = np.where(valid[:, None], x[b, tv], 0.0).astype(np.float32)
        # [NSEG, 128, DCH, SEG]: (s, p, dc, j) = xw[s*SEG + j, dc*128 + p]
        x_t = np.ascontiguousarray(
            xw.reshape(NSEG, SEG, DCH, CH).transpose(0, 3, 2, 1)).astype(np.float16)

        pos = np.where(valid, seg_pos[b, tv], 0).astype(np.float32)
        ang = pos[None, :] / timescale[:, None]
        sc = np.empty((CH, 2, KV_TOK), np.float32)
        sc[:, 0, :] = np.cos(ang)
        sc[:, 1, :] = np.sin(ang)
        sc_t = np.ascontiguousarray(sc).astype(np.float16)

        mb = mask_tok[b, t0:t0 + TQ][:, tv] & valid[None, :]
        bias_c = np.where(mb.T, 1.0, 0.0).astype(np.float32)  # [KV_TOK, TQ]
        # [NSEG, 128, SEG_CH, TQ]: (s, p, c, t) = bias_c[s*SEG + c*128 + p, t]
        bias_t = np.ascontiguousarray(
            bias_c.reshape(NSEG, SEG_CH, CH, TQ).transpose(0, 2, 1, 3)).astype(BF)

        entry = dict(b=b, t0=t0, x_t=x_t, sc_t=sc_t, bias_t=bias_t)

        if EXTRA:
            n_ex = EXTRA * CH
            mo = mask_old[b, t0:t0 + TQ]
            used_o = mo.any(axis=0)
            order = np.argsort(~used_o, kind='stable')
            sel = order[:n_ex]
            ck = np.asarray(inputs['cache_k'], dtype=np.float32)[b][old_slots[b][sel]]
            cv = np.asarray(inputs['cache_v'], dtype=np.float32)[b][old_slots[b][sel]]
            # kc: [KH, 128, HCH, n_ex]: (kh, p, hc, s) = ck[s, kh, hc*128+p]
            entry['kc_t'] = np.ascontiguousarray(
                ck.reshape(n_ex, KH, HCH, CH).transpose(1, 3, 2, 0)).astype(np.float16)
            # vc: [KH, EXTRA//SEG_CH, 128, SEG_CH, H]:
            #   (kh, sx, p, st, h) = cv[sx*SEG + st*128 + p, kh, h]
            entry['vc_t'] = np.ascontiguousarray(
                cv.reshape(EXTRA // SEG_CH, SEG_CH, CH, KH, H)
                .transpose(3, 0, 2, 1, 4)).astype(BF)
            bc_ = np.where(mo[:, sel].T, 1.0, 0.0).astype(np.float32)  # [n_ex, TQ]
            entry['biasc_t'] = np.ascontiguousarray(
                bc_.reshape(EXTRA // SEG_CH, SEG_CH, CH, TQ)
                .transpose(0, 2, 1, 3)).astype(BF)
        per_core.append(entry)

    return dict(W=W, OWN_OFF=OWN_OFF, KV_TOK=KV_TOK, NSEG=NSEG, EXTRA=EXTRA,
                s_x=1.0, per_core=per_core)


def _prep_weights(inputs, s_x=None):
    w_kv = np.asarray(inputs['w_kv'], dtype=np.float32)
    pre_attn = (1.0 + np.asarray(inputs['pre_attn_scale'], dtype=np.float32))
    pre_ffw = (1.0 + np.asarray(inputs['pre_ffw_scale'], dtype=np.float32))

    def tile_dh(w):  # [*, D, H] -> [*, 128, DCH, H]
        lead = w.shape[:-2]
        return np.ascontiguousarray(
            w.reshape(*lead, DCH, CH, H).transpose(
                *range(len(lead)), len(lead) + 1, len(lead), len(lead) + 2))

    wq = np.asarray(inputs['w_q'], dtype=np.float32) * pre_attn[None, :, None]
    wk = w_kv[0] * pre_attn[None, :, None]
    wv = w_kv[1] * pre_attn[None, :, None]

    wav = np.asarray(inputs['w_attn_vec'], dtype=np.float32)  # [N, H, D]
    # [N, 4, 128, HCH, 512]: (n, q4, p, hc, j) = wav[n, hc*128+p, q4*512+j]
    wav_t = np.ascontiguousarray(
        wav.reshape(N, HCH, CH, 4, TQ).transpose(0, 3, 2, 1, 4)).astype(BF)

    w_g = np.asarray(inputs['w_gating'], dtype=np.float32)    # [2, F, D]
    w_g_T = w_g.transpose(0, 2, 1) * pre_ffw[None, :, None]   # [2, D, F]
    # [2, 64, 128, DCH, 128]: (g, fc, p, dc, j) = w_g_T[g, dc*128+p, fc*128+j]
    wg_t = np.ascontiguousarray(
        w_g_T.reshape(2, DCH, CH, F // CH, CH).transpose(0, 3, 2, 1, 4)).astype(BF)

    w_lin = np.asarray(inputs['w_linear'], dtype=np.float32)  # [F, D]
    # [8, 4, 128, 8, 4, 128]:
    #   (fg, dcq, p, fc, dcl, j) = w_lin[(fg*8+fc)*128+p, (dcq*4+dcl)*128+j]
    wl_t = np.ascontiguousarray(
        w_lin.reshape(FGROUPS, FPG, CH, 4, 4, CH)
        .transpose(0, 3, 2, 1, 4, 5)).astype(BF)

    return dict(
        wq_t=tile_dh(wq).astype(np.float16),  # [N, 128, DCH, H]
        wk_t=tile_dh(wk).astype(np.float16),  # [KH, 128, DCH, H]
        wv_t=tile_dh(wv).astype(np.float16),
        wav_t=wav_t, wg_t=wg_t, wl_t=wl_t,
        q_scale=np.ascontiguousarray(
            (1.0 + np.asarray(inputs['q_norm_scale'], dtype=np.float32))
            .reshape(HCH, CH).T),                                 # [128, 2]
        k_scale=np.ascontiguousarray(
            (1.0 + np.asarray(inputs['k_norm_scale'], dtype=np.float32))
            .reshape(HCH, CH).T),
        post_attn=np.ascontiguousarray(
            (1.0 + np.asarray(inputs['post_attn_scale'], dtype=np.float32))
            .reshape(DCH, CH).T),                                 # [128, 16]
        post_ffw=np.ascontiguousarray(
            (1.0 + np.asarray(inputs['post_ffw_scale'], dtype=np.float32))
            .reshape(DCH, CH).T),
        skip=float(np.asarray(inputs['skip_scale']).reshape(-1)[0]),
    )


# ----------------------------------------------------------------------------
# device kernel builder
# ----------------------------------------------------------------------------

def _build_nc(W, OWN_OFF, EXTRA, skip):
    import concourse.bass as bass  # noqa: F401
    import concourse.tile as tile
    from concourse import mybir, bacc
    from contextlib import ExitStack

    F32 = mybir.dt.float32
    F32R = mybir.dt.float32r
    BF16 = mybir.dt.bfloat16
    F16 = mybir.dt.float16
    AF = mybir.ActivationFunctionType
    OP = mybir.AluOpType

    NSEG = W // SEG_CH
    OWN_SEG = OWN_OFF // SEG
    NSEG_X = EXTRA // SEG_CH if EXTRA else 0

    nc = bacc.Bacc()
    d_x = nc.declare_dram_parameter("x_t", [NSEG, CH, DCH, SEG], F16, isOutput=False)
    d_sc = nc.declare_dram_parameter("sc_t", [CH, 2, W * CH], F16, isOutput=False)
    d_bias = nc.declare_dram_parameter("bias_t", [NSEG, CH, SEG_CH, TQ], BF16, isOutput=False)
    d_wq = nc.declare_dram_parameter("wq_t", [N, CH, DCH, H], F16, isOutput=False)
    d_wk = nc.declare_dram_parameter("wk_t", [KH, CH, DCH, H], F16, isOutput=False)
    d_wv = nc.declare_dram_parameter("wv_t", [KH, CH, DCH, H], F16, isOutput=False)
    d_wav = nc.declare_dram_parameter("wav_t", [N, 4, CH, HCH, TQ], BF16, isOutput=False)
    d_wg = nc.declare_dram_parameter("wg_t", [2, F // CH, CH, DCH, CH], BF16, isOutput=False)
    d_wl = nc.declare_dram_parameter("wl_t", [FGROUPS, 4, CH, FPG, 4, CH], BF16, isOutput=False)
    d_qs = nc.declare_dram_parameter("q_scale", [CH, HCH], F32, isOutput=False)
    d_ks = nc.declare_dram_parameter("k_scale", [CH, HCH], F32, isOutput=False)
    d_pa = nc.declare_dram_parameter("post_attn", [CH, DCH], F32, isOutput=False)
    d_pf = nc.declare_dram_parameter("post_ffw", [CH, DCH], F32, isOutput=False)
    if EXTRA:
        d_kc = nc.declare_dram_parameter("kc_t", [KH, CH, HCH, EXTRA * CH], F16, isOutput=False)
        d_vc = nc.declare_dram_parameter("vc_t", [KH, NSEG_X, CH, SEG_CH, H], BF16, isOutput=False)
        d_biasc = nc.declare_dram_parameter("biasc_t", [NSEG_X, CH, SEG_CH, TQ], BF16, isOutput=False)
    d_out = nc.declare_dram_parameter("out_t", [CH, DCH, TQ], F16, isOutput=True)

    with tile.TileContext(nc) as tc, \
            nc.allow_low_precision(reason="bf16 weights / f32r activations"), \
            ExitStack() as ctx:
        cpool = ctx.enter_context(tc.tile_pool(name="const", bufs=1))
        ones_f = cpool.tile([CH, CH], F32)
        nc.vector.memset(ones_f[:], 1.0)
        ones = cpool.tile([CH, CH], F32R)
        nc.vector.tensor_copy(ones[:], ones_f[:])
        ones_b = cpool.tile([CH, 1], BF16)
        nc.vector.tensor_copy(ones_b[:], ones[:, :1])
        ones_brow = cpool.tile([1, CH], BF16)
        nc.vector.memset(ones_brow[:], 1.0)
        ones_h = cpool.tile([CH, 1], F16)
        nc.vector.tensor_copy(ones_h[:], ones[:, :1])
        eps1 = cpool.tile([1, 1], F32)
        nc.vector.memset(eps1[:], EPS)
        epsp = cpool.tile([CH, 1], F32)
        nc.vector.memset(epsp[:], EPS)
        qs_t = cpool.tile([CH, HCH], F32)
        nc.sync.dma_start(qs_t[:], d_qs[:])
        ks_t = cpool.tile([CH, HCH], F32)
        nc.sync.dma_start(ks_t[:], d_ks[:])
        pa_t = cpool.tile([CH, DCH], F32)
        nc.sync.dma_start(pa_t[:], d_pa[:])
        pf_t = cpool.tile([CH, DCH], F32)
        nc.sync.dma_start(pf_t[:], d_pf[:])
        sc_t = cpool.tile([CH, 2, W * CH], F16)
        nc.sync.dma_start(sc_t[:], d_sc[:])
        # ln(2^-40): compensates the 2^-40 pre-scale that keeps softmax sums
        # inside Ln's valid input range [-2^64, 2^64]
        ln240 = cpool.tile([1, 1], F32)
        nc.vector.memset(ln240[:], float(-40.0 * np.log(2.0)))

        ps_mm = ctx.enter_context(tc.tile_pool(name="ps_mm", bufs=6, space="PSUM"))
        ps_st = ctx.enter_context(tc.tile_pool(name="ps_st", bufs=2, space="PSUM"))
        cd = ctx.enter_context(tc.tile_pool(name="cdpool", bufs=1))

        def row_rstd(pool, stat_psum, inv_n, tag, ln_sink=None):
            """[1, n] PSUM sum-of-squares -> [1, n] f32r rstd in SBUF."""
            n = stat_psum.shape[-1]
            lnm = pool.tile([1, n], F32, tag=f"ln_{tag}", name="lnm")
            ln_i = nc.scalar.activation(out=lnm[:], in_=stat_psum[:], func=AF.Ln,
                                        bias=eps1[:], scale=inv_n)
            rstd = pool.tile([1, n], F32R, tag=f"rstd_{tag}", name="rstd")
            nc.scalar.activation(out=rstd[:], in_=lnm[:], func=AF.Exp, scale=-0.5)
            if ln_sink is not None:
                ln_sink.append(ln_i)
            return rstd

        def bcast(pool, rstd, tag, dtype=None):
            """[1, n] f32r -> [128, n] broadcast via PE + ACT copy."""
            n = rstd.shape[-1]
            bcp = ps_st.tile([CH, n], F32, tag="stat", name="bcp")
            nc.tensor.matmul(bcp[:], ones[:1, :], rstd[:], start=True, stop=True)
            bc = pool.tile([CH, n], dtype or F32, tag=f"bc_{tag}", name="bc")
            nc.scalar.activation(out=bc[:], in_=bcp[:], func=AF.Copy, scale=1.0)
            return bc

        # ==================================================================
        # attention
        # ==================================================================
        with tc.tile_pool(name="bpool", bufs=1) as bpool:
            qall = bpool.tile([CH, N, HCH, TQ], F16)      # 16KB/part
            Oall = bpool.tile([CH, N, HCH, TQ], BF16)      # 16KB/part
            sums = bpool.tile([1, N, TQ], BF16)

            # ---- KV pools open early so segment-0 x/bias/weights prefetch
            # ahead of the Q-phase DMA stream ----
            with tc.tile_pool(name="xs_s", bufs=2) as xss, \
                    tc.tile_pool(name="bias_s", bufs=2) as bss:
                xs0 = xss.tile([CH, DCH, SEG], F16, tag="xs", name="xs")
                nc.scalar.dma_start(xs0[:], d_x[OWN_SEG])
                bias0 = bss.tile([CH, SEG_CH, TQ], BF16, tag="bias", name="bias_seg")
                nc.scalar.dma_start(bias0[:], d_bias[OWN_SEG])
                xq = xs0
                cos_o = sc_t[:, 0, OWN_OFF:OWN_OFF + TQ]
                sin_o = sc_t[:, 1, OWN_OFF:OWN_OFF + TQ]
                with tc.tile_pool(name="qph1", bufs=1) as qph1, \
                        tc.tile_pool(name="qph2", bufs=2) as qph2, \
                        tc.tile_pool(name="wq_s", bufs=3) as wqs:
                    qraw = qph1.tile([CH, N, HCH, TQ], F16, tag="qraw")
                    qrow = qph1.tile([1, N * TQ], F32R, tag="qrow")
                    QB = N // 2
                    for qb in range(2):
                        for n in range(qb * QB, (qb + 1) * QB):
                            wq_t = wqs.tile([CH, DCH, H], F16, tag="wq", name="wq_t")
                            nc.sync.dma_start(wq_t[:], d_wq[n])
                            for hc in range(HCH):
                                qp = ps_mm.tile([CH, TQ], F32, tag="mm", name="qp")
                                for dc in range(DCH):
                                    nc.tensor.matmul(qp[:],
                                                     wq_t[:, dc, hc * CH:(hc + 1) * CH],
                                                     xq[:, dc, :],
                                                     start=(dc == 0), stop=(dc == DCH - 1))
                                sq = qph2.tile([CH, TQ], F16, tag="sq", name="sq")
                                nc.scalar.activation(out=sq[:], in_=qp[:], func=AF.Square, scale=1.0)
                                qst = ps_st.tile([1, TQ], F32, tag="stat", name="qst")
                                nc.tensor.matmul(qst[:], ones_h[:, :1], sq[:],
                                                 start=True, stop=True)
                                nc.vector.tensor_scalar_mul(qraw[:, n, hc, :], qp[:],
                                                            qs_t[:, hc:hc + 1])
                                if hc == 0:
                                    nc.scalar.activation(out=qrow[:, n * TQ:(n + 1) * TQ],
                                                         in_=qst[:], func=AF.Copy, scale=1.0)
                                else:
                                    nc.vector.tensor_tensor(qrow[:, n * TQ:(n + 1) * TQ],
                                                            qrow[:, n * TQ:(n + 1) * TQ],
                                                            qst[:], OP.add)
                        qsl = qrow[:, qb * QB * TQ:(qb + 1) * QB * TQ]
                        nc.scalar.activation(out=qsl, in_=qsl, func=AF.Ln,
                                             bias=eps1[:], scale=1.0 / H)
                        nc.scalar.activation(out=qsl, in_=qsl, func=AF.Exp, scale=-0.5)
                        for n in range(qb * QB, (qb + 1) * QB):
                            bc = bcast(qph2, qrow[:, n * TQ:(n + 1) * TQ], "q", dtype=F16)
                            for hc in range(HCH):
                                nc.vector.tensor_tensor(qraw[:, n, hc, :], qraw[:, n, hc, :],
                                                        bc[:], OP.mult)
                            t0_ = qph2.tile([CH, TQ], F16, tag="qt0", name="t0_")
                            t1_ = qph2.tile([CH, TQ], F16, tag="qt1", name="t1_")
                            nc.vector.tensor_tensor(t0_[:], qraw[:, n, 0, :], sin_o, OP.mult)
                            nc.vector.tensor_tensor(t1_[:], qraw[:, n, 1, :], sin_o, OP.mult)
                            nc.vector.tensor_tensor(qraw[:, n, 0, :], qraw[:, n, 0, :], cos_o, OP.mult)
                            nc.vector.tensor_tensor(qraw[:, n, 1, :], qraw[:, n, 1, :], cos_o, OP.mult)
                            nc.vector.tensor_tensor(qall[:, n, 0, :], qraw[:, n, 0, :], t1_[:], OP.subtract)
                            nc.vector.tensor_tensor(qall[:, n, 1, :], qraw[:, n, 1, :], t0_[:], OP.add)


                # ---- KV + attend, software-pipelined across segments:
                # the K/V projection + stats + normalize/rope of segment s+1
                # is emitted BEFORE the attends of segment s, so the PE's
                # attend matmuls (which wait on the tanh/exp chain) always
                # have the next segment's projection matmuls to overlap with,
                # and vice versa. ----
                kvctx = ExitStack()
                ap1 = kvctx.enter_context(tc.tile_pool(name="ap1", bufs=2))
                ap2 = kvctx.enter_context(tc.tile_pool(name="ap2", bufs=2))
                krw = kvctx.enter_context(tc.tile_pool(name="krw", bufs=2))
                ktp = kvctx.enter_context(tc.tile_pool(name="ktp", bufs=8))
                vtp = kvctx.enter_context(tc.tile_pool(name="vtp", bufs=8))
                epp = kvctx.enter_context(tc.tile_pool(name="epp", bufs=3))
                wks = kvctx.enter_context(tc.tile_pool(name="wk_s", bufs=2))

                def seg_ranges(seg):
                    """Per key-chunk query ranges implied by the sliding
                    window + causality (chunk-granular; the 0/1 bias handles
                    the exact interior masking). Ordered so the first range
                    covers the union (its psum `start` zeroes everything the
                    later sub-range matmuls accumulate into)."""
                    rngs = []
                    for st in range(SEG_CH):
                        o0 = seg * SEG + st * CH - OWN_OFF
                        q_lo = max(0, o0)
                        q_hi = min(TQ, o0 + CH - 1 + WINDOW)
                        if q_lo < q_hi:
                            rngs.append((st, q_lo, q_hi))
                    rngs.sort(key=lambda r: r[1] - r[2])
                    assert rngs and all(r[1] >= rngs[0][1] and r[2] <= rngs[0][2]
                                        for r in rngs)
                    return rngs

                FULL_RANGES = [(st, 0, TQ) for st in range(SEG_CH)]

                def attend_seg(kts, vts, bias_seg, first, ranges=None, post=None):
                    """Attend all N heads of one segment, with a one-head
                    lookahead: the logits/tanh/exp of head n are emitted
                    before the AV matmuls of head n-1, hiding the softcap
                    chain latency behind the next head's PE work."""
                    ranges = ranges or FULL_RANGES
                    _, u0, u1 = ranges[0]

                    def flush_head(n_head, eps_):
                        v_seg = vts[n_head // G]
                        for hc in range(HCH):
                            op = ps_mm.tile([CH, TQ], F32, tag="mm", name="op")
                            for i, (st, q0, q1) in enumerate(ranges):
                                nc.tensor.matmul(
                                    op[:, q0:q1], v_seg[:, st, hc * CH:(hc + 1) * CH],
                                    eps_[i][:, :q1 - q0],
                                    start=(i == 0), stop=(i == len(ranges) - 1),
                                    skip_group_check=True)
                            if first:
                                nc.scalar.activation(out=Oall[:, n_head, hc, :],
                                                     in_=op[:], func=AF.Copy, scale=1.0)
                            else:
                                nc.vector.tensor_tensor(Oall[:, n_head, hc, u0:u1],
                                                        Oall[:, n_head, hc, u0:u1],
                                                        op[:, u0:u1], OP.add)
                        sp = ps_st.tile([1, TQ], F32, tag="stat", name="sp")
                        for i, (st, q0, q1) in enumerate(ranges):
                            nc.tensor.matmul(sp[:, q0:q1], ones_b[:, :1],
                                             eps_[i][:, :q1 - q0],
                                             start=(i == 0), stop=(i == len(ranges) - 1),
                                             skip_group_check=True)
                        if first:
                            nc.scalar.activation(out=sums[:, n_head, :], in_=sp[:],
                                                 func=AF.Copy, scale=1.0)
                        else:
                            nc.vector.tensor_tensor(sums[:, n_head, u0:u1],
                                                    sums[:, n_head, u0:u1],
                                                    sp[:, u0:u1], OP.add)
                        if post is not None:
                            post(n_head)

                    prev = None
                    for n_head in range(N):
                        kT_seg = kts[n_head // G]
                        eps_ = []
                        for st, q0, q1 in ranges:
                            nq = q1 - q0
                            lg = ps_mm.tile([CH, TQ], F32, tag="mm", name="lg")
                            for hc in range(HCH):
                                nc.tensor.matmul(
                                    lg[:, :nq], kT_seg[:, hc, st * CH:(st + 1) * CH],
                                    qall[:, n_head, hc, q0:q1],
                                    start=(hc == 0), stop=(hc == HCH - 1))
                            th = epp.tile([CH, TQ], F32R, tag="tanh", name="th", bufs=2)
                            nc.scalar.activation(out=th[:, :nq], in_=lg[:, :nq],
                                                 func=AF.Tanh, scale=1.0 / SOFT_CAP)
                            ep = epp.tile([CH, TQ], BF16, tag="expp", name="ep", bufs=8)
                            nc.scalar.activation(out=ep[:, :nq], in_=th[:, :nq],
                                                 func=AF.Exp, scale=SOFT_CAP)
                            nc.vector.tensor_tensor(ep[:, :nq], ep[:, :nq],
                                                    bias_seg[:, st, q0:q1], OP.mult)
                            eps_.append(ep)
                        if prev is not None:
                            flush_head(*prev)
                        prev = (n_head, eps_)
                    flush_head(*prev)

                def project_segment(seg, dma0):
                    """K/V projection + stats + normalize + rope for one
                    segment; returns (kts, vts, bias_seg) ready to attend."""
                    ssl = slice(seg * SEG, (seg + 1) * SEG)
                    if dma0 is not None:
                        xs, bias_seg = dma0
                    else:
                        xs = xss.tile([CH, DCH, SEG], F16, tag="xs", name="xs")
                        nc.scalar.dma_start(xs[:], d_x[seg])
                        bias_seg = bss.tile([CH, SEG_CH, TQ], BF16, tag="bias", name="bias_seg")
                        nc.scalar.dma_start(bias_seg[:], d_bias[seg])
                    cos_s = sc_t[:, 0, ssl]
                    sin_s = sc_t[:, 1, ssl]

                    kts, vts = [], []
                    krow = krw.tile([1, KH * SEG], F32R, tag="krow", name="krow")
                    vst = krw.tile([CH, KH, SEG_CH], F32, tag="vst", name="vst")
                    for kh in range(KH):
                        wk_t = wks.tile([CH, DCH, H], F16, tag="wkv", name="wk_t")
                        nc.scalar.dma_start(wk_t[:], d_wk[kh])
                        kT_seg = ktp.tile([CH, HCH, SEG], F16, tag="kt", name="kT_seg")
                        for hc in range(HCH):
                            kp = ps_mm.tile([CH, SEG], F32, tag="mm", name="kp")
                            for dc in range(DCH):
                                nc.tensor.matmul(kp[:],
                                                 wk_t[:, dc, hc * CH:(hc + 1) * CH],
                                                 xs[:, dc, :],
                                                 start=(dc == 0), stop=(dc == DCH - 1))
                            ksq = ap1.tile([CH, SEG], F16, tag="sq", name="ksq")
                            nc.scalar.activation(out=ksq[:], in_=kp[:], func=AF.Square, scale=1.0)
                            kst = ps_st.tile([1, SEG], F32, tag="stat", name="kst")
                            nc.tensor.matmul(kst[:], ones_h[:, :1], ksq[:],
                                             start=True, stop=True)
                            nc.vector.tensor_scalar_mul(kT_seg[:, hc, :], kp[:],
                                                        ks_t[:, hc:hc + 1])
                            if hc == 0:
                                nc.scalar.activation(out=krow[:, kh * SEG:(kh + 1) * SEG],
                                                     in_=kst[:], func=AF.Copy, scale=1.0)
                            else:
                                nc.vector.tensor_tensor(krow[:, kh * SEG:(kh + 1) * SEG],
                                                        krow[:, kh * SEG:(kh + 1) * SEG],
                                                        kst[:], OP.add)
                        kts.append(kT_seg)

                        wv_t = wks.tile([CH, DCH, H], F16, tag="wkv", name="wv_t")
                        nc.scalar.dma_start(wv_t[:], d_wv[kh])
                        v_seg = vtp.tile([CH, SEG_CH, H], BF16, tag="vt", name="v_seg")
                        for sp2 in range(2):
                            vp = ps_mm.tile([CH, 2, H], F32, tag="mm", name="vp")
                            for sl in range(2):
                                st = sp2 * 2 + sl
                                for dc in range(DCH):
                                    nc.tensor.matmul(vp[:, sl, :],
                                                     xs[:, dc, st * CH:(st + 1) * CH],
                                                     wv_t[:, dc, :],
                                                     start=(dc == 0), stop=(dc == DCH - 1))
                            vsq = ap1.tile([CH, 2, H], F16, tag="sq", name="vsq")
                            nc.scalar.activation(out=vsq[:], in_=vp[:], func=AF.Square, scale=1.0)
                            for sl in range(2):
                                st = sp2 * 2 + sl
                                nc.vector.reduce_sum(vst[:, kh, st:st + 1], vsq[:, sl, :],
                                                     axis=mybir.AxisListType.X)
                            nc.scalar.activation(out=v_seg[:, sp2 * 2:(sp2 + 1) * 2, :],
                                                 in_=vp[:], func=AF.Copy, scale=1.0)
                        vts.append(v_seg)

                    # -- one Ln/Exp pair per segment for all k and v stats --
                    nc.scalar.activation(out=krow[:], in_=krow[:], func=AF.Ln,
                                         bias=eps1[:], scale=1.0 / H)
                    nc.scalar.activation(out=vst[:], in_=vst[:], func=AF.Ln,
                                         bias=epsp[:], scale=1.0 / H)
                    nc.scalar.activation(out=krow[:], in_=krow[:], func=AF.Exp, scale=-0.5)
                    krstd = krow
                    nc.scalar.activation(out=vst[:], in_=vst[:], func=AF.Exp, scale=-0.5)
                    vrstd = vst

                    return kts, vts, bias_seg, krstd, vrstd, cos_s, sin_s

                def project_phase2(ph1, khs=range(KH)):
                    # normalize + rope (DVE + bck broadcasts on the PE, gated
                    # on the Ln/Exp chain -- emitted AFTER the previous
                    # segment's attends so those never wait on it)
                    kts, vts, _, krstd, vrstd, cos_s, sin_s = ph1
                    for kh in khs:
                        kT_seg, v_seg = kts[kh], vts[kh]
                        bck = bcast(ap2, krstd[:, kh * SEG:(kh + 1) * SEG], "k", dtype=F16)
                        for hc in range(HCH):
                            nc.vector.tensor_tensor(kT_seg[:, hc, :], kT_seg[:, hc, :],
                                                    bck[:], OP.mult)
                        t0_ = ap2.tile([CH, SEG], F16, tag="kt0", name="t0_")
                        t1_ = ap2.tile([CH, SEG], F16, tag="kt1", name="t1_")
                        nc.vector.tensor_tensor(t0_[:], kT_seg[:, 0, :], sin_s, OP.mult)
                        nc.vector.tensor_tensor(t1_[:], kT_seg[:, 1, :], sin_s, OP.mult)
                        nc.vector.tensor_tensor(kT_seg[:, 0, :], kT_seg[:, 0, :], cos_s, OP.mult)
                        nc.vector.tensor_tensor(kT_seg[:, 1, :], kT_seg[:, 1, :], cos_s, OP.mult)
                        nc.vector.tensor_tensor(kT_seg[:, 0, :], kT_seg[:, 0, :], t1_[:], OP.subtract)
                        nc.vector.tensor_tensor(kT_seg[:, 1, :], kT_seg[:, 1, :], t0_[:], OP.add)
                        for st in range(SEG_CH):
                            nc.vector.tensor_scalar_mul(v_seg[:, st, :], v_seg[:, st, :],
                                                        vrstd[:, kh, st:st + 1])

                def normalize_head(n_head):
                    # divide Oall by the softmax sum: PE broadcast + fast
                    # DVE reciprocal, emitted per head right after its last
                    # attend so attn_vec can start while later heads attend
                    bcp = ps_st.tile([CH, TQ], F32, tag="stat", name="bcp")
                    nc.tensor.matmul(bcp[:], ones_brow[:], sums[:, n_head, :],
                                     start=True, stop=True)
                    bc = ap2.tile([CH, TQ], F32, tag="bc_s", name="bc", bufs=1)
                    nc.vector.reciprocal_approx_fast(bc[:], bcp[:])
                    for hc in range(HCH):
                        nc.vector.tensor_tensor(Oall[:, n_head, hc, :],
                                                Oall[:, n_head, hc, :], bc[:], OP.mult)

                seg_order = [OWN_SEG] + [s for s in range(NSEG) if s != OWN_SEG]
                pend = None
                for idx, seg in enumerate(seg_order):
                    dma0 = (xs0, bias0) if idx == 0 else None
                    ph1 = project_segment(seg, dma0)
                    if pend is not None:
                        # normalize head group 0 first so the NEXT iteration's
                        # attends find it ready without queueing behind this
                        # iteration's attend DVE work
                        project_phase2(ph1, range(0, 1))
                        attend_seg(*pend)
                        project_phase2(ph1, range(1, KH))
                    else:
                        project_phase2(ph1)
                    pend = (ph1[0], ph1[1], ph1[2], idx == 0, seg_ranges(seg),
                            None)
                kts_p, vts_p, bias_p, first_p, rng_p, _ = pend
                attend_seg(kts_p, vts_p, bias_p, first_p, rng_p,
                           post=None if NSEG_X else normalize_head)

                for sx in range(NSEG_X):
                    bias_seg = bss.tile([CH, SEG_CH, TQ], BF16, tag="bias", name="bias_seg")
                    nc.sync.dma_start(bias_seg[:], d_biasc[sx])
                    kts_x, vts_x = [], []
                    for kh in range(KH):
                        kT_seg = ktp.tile([CH, HCH, SEG], F16, tag="kt", name="kT_seg")
                        nc.sync.dma_start(kT_seg[:],
                                          d_kc[kh][:, :, sx * SEG:(sx + 1) * SEG])
                        v_seg = vtp.tile([CH, SEG_CH, H], BF16, tag="vt", name="v_seg")
                        nc.sync.dma_start(v_seg[:], d_vc[kh, sx])
                        kts_x.append(kT_seg)
                        vts_x.append(v_seg)
                    attend_seg(kts_x, vts_x, bias_seg, False, None,
                               post=normalize_head if sx == NSEG_X - 1 else None)

                kvctx.close()

            # ---- normalize O (divide by softmax sum), then attn_vec ----
            with tc.tile_pool(name="avacc", bufs=1) as avaccp:
              with tc.tile_pool(name="avp", bufs=3) as avpool, \
                    tc.tile_pool(name="wav_s", bufs=8) as wavs:
                attn_acc = avaccp.tile([CH, DCH, TQ], F32)
                pa_stat = ps_st.tile([1, TQ], F32, tag="stat", name="pa_stat")
                for dcq in range(4):
                    wav_ts = []
                    for n_head in range(N):
                        wav_t = wavs.tile([CH, HCH, TQ], BF16, tag="wavf", name="wav_t")
                        nc.sync.dma_start(wav_t[:], d_wav[n_head, dcq])
                        wav_ts.append(wav_t)
                    for dcl in range(4):
                        dc = dcq * 4 + dcl
                        # two half-groups (heads 0-3 / 4-7): the first half's
                        # matmuls start as soon as 4 heads are normalized
                        avp_a = ps_mm.tile([CH, TQ], F32, tag="mm", name="avp_a")
                        i = 0
                        for n_head in range(N // 2):
                            for hc in range(HCH):
                                nc.tensor.matmul(avp_a[:],
                                                 wav_ts[n_head][:, hc, dcl * CH:(dcl + 1) * CH],
                                                 Oall[:, n_head, hc, :],
                                                 start=(i == 0), stop=(i == N - 1))
                                i += 1
                        avp_b = ps_mm.tile([CH, TQ], F32, tag="mm", name="avp_b")
                        i = 0
                        for n_head in range(N // 2, N):
                            for hc in range(HCH):
                                nc.tensor.matmul(avp_b[:],
                                                 wav_ts[n_head][:, hc, dcl * CH:(dcl + 1) * CH],
                                                 Oall[:, n_head, hc, :],
                                                 start=(i == 0), stop=(i == N - 1))
                                i += 1
                        nc.scalar.activation(out=attn_acc[:, dc, :], in_=avp_a[:],
                                             func=AF.Copy, scale=1.0)
                        nc.vector.tensor_tensor(attn_acc[:, dc, :], attn_acc[:, dc, :],
                                                avp_b[:], OP.add)
                        sqa = avpool.tile([CH, TQ], F32R, tag="sqa", name="sqa")
                        nc.scalar.activation(out=sqa[:], in_=attn_acc[:, dc, :],
                                             func=AF.Square, scale=1.0)
                        nc.tensor.matmul(pa_stat[:], ones[:, :1], sqa[:],
                                         start=(dc == 0), stop=(dc == DCH - 1))

              # ---- post-attn norm + residual; pre-ffw norm ----
              if True:
                attn_out = cd.tile([CH, DCH, TQ], F16)
                ffw_in = cd.tile([CH, DCH, TQ], BF16)
                with tc.tile_pool(name="phc", bufs=1) as pc1, \
                        tc.tile_pool(name="phc2", bufs=2) as pc2:
                    x_own = pc1.tile([CH, DCH, SEG], F16, tag="xown")
                    nc.sync.dma_start(x_own[:], d_x[OWN_SEG])
                    rstd_pa = row_rstd(pc2, pa_stat, 1.0 / D, "pa")
                    bc = bcast(pc2, rstd_pa, "pa")
                    pf_stat = ps_st.tile([1, TQ], F32, tag="stat", name="pf_stat")
                    for dc in range(DCH):
                        tt = pc2.tile([CH, TQ], F32, tag="catmp", name="tt")
                        nc.vector.scalar_tensor_tensor(tt[:], attn_acc[:, dc, :],
                                                       pa_t[:, dc:dc + 1], bc[:],
                                                       OP.mult, OP.mult)
                        nc.vector.scalar_tensor_tensor(attn_out[:, dc, :],
                                                       x_own[:, dc, :], float(skip),
                                                       tt[:], OP.mult, OP.add)
                        sqf = pc2.tile([CH, TQ], F32R, tag="sqf", name="sqf")
                        nc.scalar.activation(out=sqf[:], in_=attn_out[:, dc, :],
                                             func=AF.Square, scale=1.0)
                        nc.tensor.matmul(pf_stat[:], ones[:, :1], sqf[:],
                                         start=(dc == 0), stop=(dc == DCH - 1))
                    rstd_pf = row_rstd(pc2, pf_stat, 1.0 / D, "pf")
                    bc2 = bcast(pc2, rstd_pf, "pf")
                    for dc in range(DCH):
                        nc.vector.tensor_tensor(ffw_in[:, dc, :], attn_out[:, dc, :], bc2[:], OP.mult)


        # ==================================================================
        # FFN
        # ==================================================================
        if True:
            with tc.tile_pool(name="dp1", bufs=1) as dp1, \
                    tc.tile_pool(name="dp2", bufs=2) as dp2, \
                    tc.tile_pool(name="actp", bufs=FHALF) as actp, \
                    tc.tile_pool(name="wg_s", bufs=4) as wgs, \
                    tc.tile_pool(name="wl_s", bufs=4) as wls:
                ffw_acc = dp1.tile([CH, DCH, TQ], F32)
                of_stat = None
                for half in range(2):
                    acts = []
                    for fgl in range(FHALF):
                        fg = half * FHALF + fgl
                        act = actp.tile([CH, FPG, TQ], BF16, tag="act", name=f"act{fgl}")
                        for fc in range(FPG):
                            fglob = fg * FPG + fc
                            gp = []
                            for g01 in range(2):
                                wg_t = wgs.tile([CH, DCH, CH], BF16, tag="wg", name="wg_t")
                                nc.sync.dma_start(wg_t[:], d_wg[g01, fglob])
                                pg = ps_mm.tile([CH, TQ], F32, tag="mm", name=f"pg{g01}")
                                for dc in range(DCH):
                                    nc.tensor.matmul(pg[:], wg_t[:, dc, :], ffw_in[:, dc, :],
                                                     start=(dc == 0), stop=(dc == DCH - 1))
                                gp.append(pg)
                            gel = dp2.tile([CH, TQ], F32, tag="gel", name="gel")
                            nc.scalar.activation(out=gel[:], in_=gp[0][:],
                                                 func=AF.Gelu_apprx_tanh, scale=1.0)
                            nc.vector.tensor_tensor(act[:, fc, :], gel[:], gp[1][:], OP.mult)
                        acts.append(act)
                    for dcq in range(4):
                        wl_ts = []
                        for fgl in range(FHALF):
                            fg = half * FHALF + fgl
                            wl_t = wls.tile([CH, FPG, 4, CH], BF16, tag="wl", name="wl_t")
                            nc.sync.dma_start(wl_t[:], d_wl[fg, dcq])
                            wl_ts.append(wl_t)
                        for dcl in range(4):
                            dc = dcq * 4 + dcl
                            pf = ps_mm.tile([CH, TQ], F32, tag="mm", name="pf")
                            i = 0
                            for fgl in range(FHALF):
                                for fc in range(FPG):
                                    nc.tensor.matmul(pf[:], wl_ts[fgl][:, fc, dcl, :],
                                                     acts[fgl][:, fc, :],
                                                     start=(i == 0), stop=(i == FHALF * FPG - 1))
                                    i += 1
                            if half == 0:
                                nc.scalar.activation(out=ffw_acc[:, dc, :], in_=pf[:],
                                                     func=AF.Copy, scale=1.0)
                            else:
                                nc.vector.tensor_tensor(ffw_acc[:, dc, :], ffw_acc[:, dc, :],
                                                        pf[:], OP.add)
                                sqo = dp2.tile([CH, TQ], F32R, tag="sqo", name="sqo")
                                nc.scalar.activation(out=sqo[:], in_=ffw_acc[:, dc, :],
                                                     func=AF.Square, scale=1.0)
                                if of_stat is None:
                                    of_stat = ps_st.tile([1, TQ], F32, tag="stat", name="of_stat")
                                nc.tensor.matmul(of_stat[:], ones[:, :1], sqo[:],
                                                 start=(dc == 0), stop=(dc == DCH - 1))
                # post-ffw norm + final residual (into attn_out, then store)
                rstd_of = row_rstd(dp2, of_stat, 1.0 / D, "of")
                bc3 = bcast(dp2, rstd_of, "of")
                for dc in range(DCH):
                    tt = dp2.tile([CH, TQ], F32, tag="fftmp", name="tt")
                    nc.vector.scalar_tensor_tensor(tt[:], ffw_acc[:, dc, :],
                                                   pf_t[:, dc:dc + 1], bc3[:],
                                                   OP.mult, OP.mult)
                    nc.vector.tensor_tensor(attn_out[:, dc, :], attn_out[:, dc, :], tt[:], OP.add)
                nc.sync.dma_start(d_out[:], attn_out[:])

    nc.finalize()
    return nc


_NC_CACHE = {}


def _in_maps(plan, w):
    in_maps = []
    for c in range(N_CORES):
        e = plan['per_core'][c]
        m = dict(x_t=e['x_t'], sc_t=e['sc_t'], bias_t=e['bias_t'],
                 wq_t=w['wq_t'], wk_t=w['wk_t'], wv_t=w['wv_t'],
                 wav_t=w['wav_t'], wg_t=w['wg_t'], wl_t=w['wl_t'],
                 q_scale=w['q_scale'], k_scale=w['k_scale'],
                 post_attn=w['post_attn'], post_ffw=w['post_ffw'])
        if plan['EXTRA']:
            m.update(kc_t=e['kc_t'], vc_t=e['vc_t'], biasc_t=e['biasc_t'])
        in_maps.append(m)
    return in_maps


def _nc_key(plan, w):
    return (plan['W'], plan['OWN_OFF'], plan['EXTRA'], w['skip'])


def kernel(**inputs) -> np.ndarray:
    from concourse.bass_utils import run_bass_kernel_spmd

    plan = _plan(inputs)
    w = _prep_weights(inputs)
    key = _nc_key(plan, w)
    if key not in _NC_CACHE:
        _NC_CACHE[key] = _build_nc(*key)
    nc = _NC_CACHE[key]

    res = run_bass_kernel_spmd(nc, _in_maps(plan, w), core_ids=list(range(N_CORES)))

    out = np.zeros((B, T, D), np.float32)
    for c in range(N_CORES):
        e = plan['per_core'][c]
        o = res.results[c]['out_t']          # [128, DCH, TQ]
        out[e['b'], e['t0']:e['t0'] + TQ] = o.transpose(2, 1, 0).reshape(TQ, D)
    return out



# revision 49
# speedup vs baseline: 1.0243x; 1.0189x over previous
"""Trainium2 Bass kernel for nn_Block_16441134809284 (sparse_attention block).

Self-contained: token-parallel over 8 NeuronCores (2 batches x 4 slices of 512
tokens). Each core computes its 512 output tokens end-to-end; KV for the
sliding window is recomputed per core from a zero-padded token window, so no
collectives are needed.

v3 design notes (on top of v2):
- Software-pipelined segment schedule: K/V projection matmuls + stats of
  segment s+1 are emitted before the attends of segment s, and the
  normalize/rope (which gates on the per-segment stats Ln/Exp) is emitted
  after them, so the PE never queues behind the rstd chain.
- attend processes all 8 heads with a one-head lookahead (logits/softcap of
  head n before the AV matmuls of head n-1), and the final segment's
  attends interleave the per-head 1/softmax-sum normalize (PE broadcast +
  fast-approx DVE reciprocal) so attn_vec starts while later heads attend.
- Chunk-granular triangular skip: per key-chunk query ranges implied by the
  sliding window + causality cut ~25% of the logits/softcap/AV work; the
  first processed range covers the union so the PSUM `start` zeroing stays
  valid.
- The pre-attention rms_norm(x) is absorbed by the q/k/v rms_norms (all three
  normalize per token after the projection, and rms_norm is scale-invariant),
  so q/k/v are projected directly from x with (1+pre_attn_scale) folded into
  the weights.
- All large tensors ship as bf16, pre-tiled partition-major so every DMA is
  128 contiguous per-partition chunks (large descriptors). Activations stay
  fp32 (float32r) except where noted; matmuls mix bf16 weights (stationary)
  with f32r moving operands, which the PE allows.
- rstd = exp(-0.5*ln(mean+eps)) on ScalarE: avoids the banned Rsqrt, the slow
  DVE reciprocal, and keeps the attention phase within two ACT table sets
  (exp_and_others / natural_log_exp_and_others).
- Softmax without max-subtraction (logits are soft-capped to +-50); masking is
  a 0/1 multiplier on exp values.
"""
import sys

for _p in ("/opt/trn_rl_repo", "/root/.axon_site/_ro/trn_rl_repo"):
    if _p not in sys.path:
        sys.path.insert(0, _p)

import numpy as np
import ml_dtypes

BF = ml_dtypes.bfloat16

K_MASK = -2.3819763e+38
SOFT_CAP = 50.0
WINDOW = 1024
ROPE_BASE = 10000.0

B, T, D, N, KH, H, F = 2, 2048, 2048, 8, 4, 256, 8192
CACHE = 4096
N_CORES = 8
SLICES = N_CORES // B
TQ = T // SLICES          # 512
EPS = 1e-6
CH = 128                  # chunk (partition) size
DCH = D // CH             # 16
HCH = H // CH             # 2
SEG_CH = 4                # kv chunks per attention segment (512 tokens)
SEG = SEG_CH * CH         # 512
FPG = 8                   # F-chunks per FFN group
FGROUPS = F // CH // FPG  # 8
FHALF = FGROUPS // 2      # 4
G = N // KH               # 2 query heads per kv head


# ----------------------------------------------------------------------------
# host-side planning
# ----------------------------------------------------------------------------

def _plan(inputs):
    attn_mask = np.asarray(inputs['attn_mask'])
    seg_pos = np.asarray(inputs['segment_pos']).astype(np.int64)
    cache_pos_in = np.asarray(inputs['cache_positions']).astype(np.int64)
    end_index = np.asarray(inputs['end_index']).astype(np.int64)
    x = np.asarray(inputs['x'], dtype=np.float32)

    slot_of_tok = (end_index[:, None] + np.arange(T)[None, :]) % CACHE
    old_slots = (end_index[:, None] + T + np.arange(CACHE - T)[None, :]) % CACHE

    cache_pos = cache_pos_in.copy()
    bidx = np.arange(B)[:, None]
    cache_pos[bidx, slot_of_tok] = seg_pos
    sliding = (cache_pos[:, None, :] > seg_pos[:, :, None] - WINDOW) & \
              (cache_pos[:, None, :] < seg_pos[:, :, None] + WINDOW)
    mask = attn_mask & sliding                      # [B, T(query), S(slot)]

    mask_tok = np.take_along_axis(mask, slot_of_tok[:, None, :], axis=2)
    mask_old = np.take_along_axis(mask, old_slots[:, None, :], axis=2)

    L_need = R_need = 0
    cache_chunks_needed = 0
    n_old = CACHE - T
    for b in range(B):
        for s in range(SLICES):
            t0 = s * TQ
            used = mask_tok[b, t0:t0 + TQ].any(axis=0)
            cidx = np.nonzero(used.reshape(T // CH, CH).any(axis=1))[0]
            if len(cidx):
                L_need = max(L_need, t0 // CH - int(cidx[0]))
                R_need = max(R_need, int(cidx[-1]) + 1 - (t0 + TQ) // CH)
            used_o = mask_old[b, t0:t0 + TQ].any(axis=0)
            co = np.nonzero(used_o.reshape(n_old // CH, CH).any(axis=1))[0]
            cache_chunks_needed = max(cache_chunks_needed, len(co))

    L_need = ((max(L_need, 0) + SEG_CH - 1) // SEG_CH) * SEG_CH
    R_need = max(R_need, 0)
    W = L_need + TQ // CH + R_need
    W = ((W + SEG_CH - 1) // SEG_CH) * SEG_CH
    OWN_OFF = L_need * CH
    KV_TOK = W * CH
    NSEG = W // SEG_CH
    EXTRA = ((cache_chunks_needed + SEG_CH - 1) // SEG_CH) * SEG_CH \
        if cache_chunks_needed else 0

    per_core = []
    frac = 2.0 * np.arange(H // 2, dtype=np.float32) / np.float32(H)
    timescale = np.float32(ROPE_BASE) ** frac
    for c in range(N_CORES):
        b, s = divmod(c, SLICES)
        t0 = s * TQ
        toks = np.arange(t0 - OWN_OFF, t0 - OWN_OFF + KV_TOK)
        valid = (toks >= 0) & (toks < T)
        tv = np.clip(toks, 0, T - 1)

        xw = np.where(valid[:, None], x[b, tv], 0.0).astype(np.float32)
        # [NSEG, 128, DCH, SEG]: (s, p, dc, j) = xw[s*SEG + j, dc*128 + p]
        x_t = np.ascontiguousarray(
            xw.reshape(NSEG, SEG, DCH, CH).transpose(0, 3, 2, 1)).astype(np.float16)

        pos = np.where(valid, seg_pos[b, tv], 0).astype(np.float32)
        ang = pos[None, :] / timescale[:, None]
        sc = np.empty((CH, 2, KV_TOK), np.float32)
        sc[:, 0, :] = np.cos(ang)
        sc[:, 1, :] = np.sin(ang)
        sc_t = np.ascontiguousarray(sc).astype(np.float16)

        mb = mask_tok[b, t0:t0 + TQ][:, tv] & valid[None, :]
        bias_c = np.where(mb.T, 1.0, 0.0).astype(np.float32)  # [KV_TOK, TQ]
        # [NSEG, 128, SEG_CH, TQ]: (s, p, c, t) = bias_c[s*SEG + c*128 + p, t]
        bias_t = np.ascontiguousarray(
            bias_c.reshape(NSEG, SEG_CH, CH, TQ).transpose(0, 2, 1, 3)).astype(BF)

        entry = dict(b=b, t0=t0, x_t=x_t, sc_t=sc_t, bias_t=bias_t)

        if EXTRA:
            n_ex = EXTRA * CH
            mo = mask_old[b, t0:t0 + TQ]
            used_o = mo.any(axis=0)
            order = np.argsort(~used_o, kind='stable')
            sel = order[:n_ex]
            ck = np.asarray(inputs['cache_k'], dtype=np.float32)[b][old_slots[b][sel]]
            cv = np.asarray(inputs['cache_v'], dtype=np.float32)[b][old_slots[b][sel]]
            # kc: [KH, 128, HCH, n_ex]: (kh, p, hc, s) = ck[s, kh, hc*128+p]
            entry['kc_t'] = np.ascontiguousarray(
                ck.reshape(n_ex, KH, HCH, CH).transpose(1, 3, 2, 0)).astype(np.float16)
            # vc: [KH, EXTRA//SEG_CH, 128, SEG_CH, H]:
            #   (kh, sx, p, st, h) = cv[sx*SEG + st*128 + p, kh, h]
            entry['vc_t'] = np.ascontiguousarray(
                cv.reshape(EXTRA // SEG_CH, SEG_CH, CH, KH, H)
                .transpose(3, 0, 2, 1, 4)).astype(BF)
            bc_ = np.where(mo[:, sel].T, 1.0, 0.0).astype(np.float32)  # [n_ex, TQ]
            entry['biasc_t'] = np.ascontiguousarray(
                bc_.reshape(EXTRA // SEG_CH, SEG_CH, CH, TQ)
                .transpose(0, 2, 1, 3)).astype(BF)
        per_core.append(entry)

    return dict(W=W, OWN_OFF=OWN_OFF, KV_TOK=KV_TOK, NSEG=NSEG, EXTRA=EXTRA,
                s_x=1.0, per_core=per_core)


def _prep_weights(inputs, s_x=None):
    w_kv = np.asarray(inputs['w_kv'], dtype=np.float32)
    pre_attn = (1.0 + np.asarray(inputs['pre_attn_scale'], dtype=np.float32))
    pre_ffw = (1.0 + np.asarray(inputs['pre_ffw_scale'], dtype=np.float32))

    def tile_dh(w):  # [*, D, H] -> [*, 128, DCH, H]
        lead = w.shape[:-2]
        return np.ascontiguousarray(
            w.reshape(*lead, DCH, CH, H).transpose(
                *range(len(lead)), len(lead) + 1, len(lead), len(lead) + 2))

    wq = np.asarray(inputs['w_q'], dtype=np.float32) * pre_attn[None, :, None]
    wk = w_kv[0] * pre_attn[None, :, None]
    wv = w_kv[1] * pre_attn[None, :, None]

    wav = np.asarray(inputs['w_attn_vec'], dtype=np.float32)  # [N, H, D]
    # [N, 4, 128, HCH, 512]: (n, q4, p, hc, j) = wav[n, hc*128+p, q4*512+j]
    wav_t = np.ascontiguousarray(
        wav.reshape(N, HCH, CH, 4, TQ).transpose(0, 3, 2, 1, 4)).astype(BF)

    w_g = np.asarray(inputs['w_gating'], dtype=np.float32)    # [2, F, D]
    w_g_T = w_g.transpose(0, 2, 1) * pre_ffw[None, :, None]   # [2, D, F]
    # [2, 64, 128, DCH, 128]: (g, fc, p, dc, j) = w_g_T[g, dc*128+p, fc*128+j]
    wg_t = np.ascontiguousarray(
        w_g_T.reshape(2, DCH, CH, F // CH, CH).transpose(0, 3, 2, 1, 4)).astype(BF)

    w_lin = np.asarray(inputs['w_linear'], dtype=np.float32)  # [F, D]
    # [8, 4, 128, 8, 4, 128]:
    #   (fg, dcq, p, fc, dcl, j) = w_lin[(fg*8+fc)*128+p, (dcq*4+dcl)*128+j]
    wl_t = np.ascontiguousarray(
        w_lin.reshape(FGROUPS, FPG, CH, 4, 4, CH)
        .transpose(0, 3, 2, 1, 4, 5)).astype(BF)

    return dict(
        wq_t=tile_dh(wq).astype(np.float16),  # [N, 128, DCH, H]
        wk_t=tile_dh(wk).astype(np.float16),  # [KH, 128, DCH, H]
        wv_t=tile_dh(wv).astype(np.float16),
        wav_t=wav_t, wg_t=wg_t, wl_t=wl_t,
        q_scale=np.ascontiguousarray(
            (1.0 + np.asarray(inputs['q_norm_scale'], dtype=np.float32))
            .reshape(HCH, CH).T),                                 # [128, 2]
        k_scale=np.ascontiguousarray(
            (1.0 + np.asarray(inputs['k_norm_scale'], dtype=np.float32))
            .reshape(HCH, CH).T),
        post_attn=np.ascontiguousarray(
            (1.0 + np.asarray(inputs['post_attn_scale'], dtype=np.float32))
            .reshape(DCH, CH).T),                                 # [128, 16]
        post_ffw=np.ascontiguousarray(
            (1.0 + np.asarray(inputs['post_ffw_scale'], dtype=np.float32))
            .reshape(DCH, CH).T),
        skip=float(np.asarray(inputs['skip_scale']).reshape(-1)[0]),
    )


# ----------------------------------------------------------------------------
# device kernel builder
# ----------------------------------------------------------------------------

def _build_nc(W, OWN_OFF, EXTRA, skip):
    import concourse.bass as bass  # noqa: F401
    import concourse.tile as tile
    from concourse import mybir, bacc
    from contextlib import ExitStack

    F32 = mybir.dt.float32
    F32R = mybir.dt.float32r
    BF16 = mybir.dt.bfloat16
    F16 = mybir.dt.float16
    AF = mybir.ActivationFunctionType
    OP = mybir.AluOpType

    NSEG = W // SEG_CH
    OWN_SEG = OWN_OFF // SEG
    NSEG_X = EXTRA // SEG_CH if EXTRA else 0

    nc = bacc.Bacc()
    d_x = nc.declare_dram_parameter("x_t", [NSEG, CH, DCH, SEG], F16, isOutput=False)
    d_sc = nc.declare_dram_parameter("sc_t", [CH, 2, W * CH], F16, isOutput=False)
    d_bias = nc.declare_dram_parameter("bias_t", [NSEG, CH, SEG_CH, TQ], BF16, isOutput=False)
    d_wq = nc.declare_dram_parameter("wq_t", [N, CH, DCH, H], F16, isOutput=False)
    d_wk = nc.declare_dram_parameter("wk_t", [KH, CH, DCH, H], F16, isOutput=False)
    d_wv = nc.declare_dram_parameter("wv_t", [KH, CH, DCH, H], F16, isOutput=False)
    d_wav = nc.declare_dram_parameter("wav_t", [N, 4, CH, HCH, TQ], BF16, isOutput=False)
    d_wg = nc.declare_dram_parameter("wg_t", [2, F // CH, CH, DCH, CH], BF16, isOutput=False)
    d_wl = nc.declare_dram_parameter("wl_t", [FGROUPS, 4, CH, FPG, 4, CH], BF16, isOutput=False)
    d_qs = nc.declare_dram_parameter("q_scale", [CH, HCH], F32, isOutput=False)
    d_ks = nc.declare_dram_parameter("k_scale", [CH, HCH], F32, isOutput=False)
    d_pa = nc.declare_dram_parameter("post_attn", [CH, DCH], F32, isOutput=False)
    d_pf = nc.declare_dram_parameter("post_ffw", [CH, DCH], F32, isOutput=False)
    if EXTRA:
        d_kc = nc.declare_dram_parameter("kc_t", [KH, CH, HCH, EXTRA * CH], F16, isOutput=False)
        d_vc = nc.declare_dram_parameter("vc_t", [KH, NSEG_X, CH, SEG_CH, H], BF16, isOutput=False)
        d_biasc = nc.declare_dram_parameter("biasc_t", [NSEG_X, CH, SEG_CH, TQ], BF16, isOutput=False)
    d_out = nc.declare_dram_parameter("out_t", [CH, DCH, TQ], F16, isOutput=True)

    with tile.TileContext(nc) as tc, \
            nc.allow_low_precision(reason="bf16 weights / f32r activations"), \
            ExitStack() as ctx:
        cpool = ctx.enter_context(tc.tile_pool(name="const", bufs=1))
        ones_f = cpool.tile([CH, CH], F32)
        nc.vector.memset(ones_f[:], 1.0)
        ones = cpool.tile([CH, CH], F32R)
        nc.vector.tensor_copy(ones[:], ones_f[:])
        ones_b = cpool.tile([CH, 1], BF16)
        nc.vector.tensor_copy(ones_b[:], ones[:, :1])
        ones_brow = cpool.tile([1, CH], BF16)
        nc.vector.memset(ones_brow[:], 1.0)
        ones_h = cpool.tile([CH, 1], F16)
        nc.vector.tensor_copy(ones_h[:], ones[:, :1])
        eps1 = cpool.tile([1, 1], F32)
        nc.vector.memset(eps1[:], EPS)
        epsp = cpool.tile([CH, 1], F32)
        nc.vector.memset(epsp[:], EPS)
        qs_t = cpool.tile([CH, HCH], F32)
        nc.gpsimd.dma_start(qs_t[:], d_qs[:])
        ks_t = cpool.tile([CH, HCH], F32)
        nc.gpsimd.dma_start(ks_t[:], d_ks[:])
        pa_t = cpool.tile([CH, DCH], F32)
        nc.gpsimd.dma_start(pa_t[:], d_pa[:])
        pf_t = cpool.tile([CH, DCH], F32)
        nc.gpsimd.dma_start(pf_t[:], d_pf[:])
        sc_t = cpool.tile([CH, 2, W * CH], F16)
        nc.gpsimd.dma_start(sc_t[:], d_sc[:])
        # ln(2^-40): compensates the 2^-40 pre-scale that keeps softmax sums
        # inside Ln's valid input range [-2^64, 2^64]
        ln240 = cpool.tile([1, 1], F32)
        nc.vector.memset(ln240[:], float(-40.0 * np.log(2.0)))

        ps_mm = ctx.enter_context(tc.tile_pool(name="ps_mm", bufs=6, space="PSUM"))
        ps_st = ctx.enter_context(tc.tile_pool(name="ps_st", bufs=2, space="PSUM"))
        cd = ctx.enter_context(tc.tile_pool(name="cdpool", bufs=1))

        def row_rstd(pool, stat_psum, inv_n, tag, ln_sink=None):
            """[1, n] PSUM sum-of-squares -> [1, n] f32r rstd in SBUF."""
            n = stat_psum.shape[-1]
            lnm = pool.tile([1, n], F32, tag=f"ln_{tag}", name="lnm")
            ln_i = nc.scalar.activation(out=lnm[:], in_=stat_psum[:], func=AF.Ln,
                                        bias=eps1[:], scale=inv_n)
            rstd = pool.tile([1, n], F32R, tag=f"rstd_{tag}", name="rstd")
            nc.scalar.activation(out=rstd[:], in_=lnm[:], func=AF.Exp, scale=-0.5)
            if ln_sink is not None:
                ln_sink.append(ln_i)
            return rstd

        def bcast(pool, rstd, tag, dtype=None):
            """[1, n] f32r -> [128, n] broadcast via PE + ACT copy."""
            n = rstd.shape[-1]
            bcp = ps_st.tile([CH, n], F32, tag="stat", name="bcp")
            nc.tensor.matmul(bcp[:], ones[:1, :], rstd[:], start=True, stop=True)
            bc = pool.tile([CH, n], dtype or F32, tag=f"bc_{tag}", name="bc")
            nc.scalar.activation(out=bc[:], in_=bcp[:], func=AF.Copy, scale=1.0)
            return bc

        # ==================================================================
        # attention
        # ==================================================================
        with tc.tile_pool(name="bpool", bufs=1) as bpool:
            qall = bpool.tile([CH, N, HCH, TQ], F16)      # 16KB/part
            Oall = bpool.tile([CH, N, HCH, TQ], BF16)      # 16KB/part
            sums = bpool.tile([1, N, TQ], BF16)

            # ---- KV pools open early so segment-0 x/bias/weights prefetch
            # ahead of the Q-phase DMA stream ----
            with tc.tile_pool(name="xs_s", bufs=2) as xss, \
                    tc.tile_pool(name="bias_s", bufs=2) as bss:
                xs0 = xss.tile([CH, DCH, SEG], F16, tag="xs", name="xs")
                nc.scalar.dma_start(xs0[:], d_x[OWN_SEG])
                bias0 = bss.tile([CH, SEG_CH, TQ], BF16, tag="bias", name="bias_seg")
                nc.scalar.dma_start(bias0[:], d_bias[OWN_SEG])
                xq = xs0
                cos_o = sc_t[:, 0, OWN_OFF:OWN_OFF + TQ]
                sin_o = sc_t[:, 1, OWN_OFF:OWN_OFF + TQ]
                with tc.tile_pool(name="qph1", bufs=1) as qph1, \
                        tc.tile_pool(name="qph2", bufs=2) as qph2, \
                        tc.tile_pool(name="wq_s", bufs=3) as wqs:
                    qraw = qph1.tile([CH, N, HCH, TQ], F16, tag="qraw")
                    qrow = qph1.tile([1, N * TQ], F32R, tag="qrow")
                    QB = N // 2
                    for qb in range(2):
                        for n in range(qb * QB, (qb + 1) * QB):
                            wq_t = wqs.tile([CH, DCH, H], F16, tag="wq", name="wq_t")
                            nc.sync.dma_start(wq_t[:], d_wq[n])
                            for hc in range(HCH):
                                qp = ps_mm.tile([CH, TQ], F32, tag="mm", name="qp")
                                for dc in range(DCH):
                                    nc.tensor.matmul(qp[:],
                                                     wq_t[:, dc, hc * CH:(hc + 1) * CH],
                                                     xq[:, dc, :],
                                                     start=(dc == 0), stop=(dc == DCH - 1))
                                sq = qph2.tile([CH, TQ], F16, tag="sq", name="sq")
                                nc.scalar.activation(out=sq[:], in_=qp[:], func=AF.Square, scale=1.0)
                                qst = ps_st.tile([1, TQ], F32, tag="stat", name="qst")
                                nc.tensor.matmul(qst[:], ones_h[:, :1], sq[:],
                                                 start=True, stop=True)
                                nc.vector.tensor_scalar_mul(qraw[:, n, hc, :], qp[:],
                                                            qs_t[:, hc:hc + 1])
                                if hc == 0:
                                    nc.scalar.activation(out=qrow[:, n * TQ:(n + 1) * TQ],
                                                         in_=qst[:], func=AF.Copy, scale=1.0)
                                else:
                                    nc.vector.tensor_tensor(qrow[:, n * TQ:(n + 1) * TQ],
                                                            qrow[:, n * TQ:(n + 1) * TQ],
                                                            qst[:], OP.add)
                        qsl = qrow[:, qb * QB * TQ:(qb + 1) * QB * TQ]
                        nc.scalar.activation(out=qsl, in_=qsl, func=AF.Ln,
                                             bias=eps1[:], scale=1.0 / H)
                        nc.scalar.activation(out=qsl, in_=qsl, func=AF.Exp, scale=-0.5)
                        for n in range(qb * QB, (qb + 1) * QB):
                            bc = bcast(qph2, qrow[:, n * TQ:(n + 1) * TQ], "q", dtype=F16)
                            for hc in range(HCH):
                                nc.vector.tensor_tensor(qraw[:, n, hc, :], qraw[:, n, hc, :],
                                                        bc[:], OP.mult)
                            t0_ = qph2.tile([CH, TQ], F16, tag="qt0", name="t0_")
                            t1_ = qph2.tile([CH, TQ], F16, tag="qt1", name="t1_")
                            nc.vector.tensor_tensor(t0_[:], qraw[:, n, 0, :], sin_o, OP.mult)
                            nc.vector.tensor_tensor(t1_[:], qraw[:, n, 1, :], sin_o, OP.mult)
                            nc.vector.tensor_tensor(qraw[:, n, 0, :], qraw[:, n, 0, :], cos_o, OP.mult)
                            nc.vector.tensor_tensor(qraw[:, n, 1, :], qraw[:, n, 1, :], cos_o, OP.mult)
                            nc.vector.tensor_tensor(qall[:, n, 0, :], qraw[:, n, 0, :], t1_[:], OP.subtract)
                            nc.vector.tensor_tensor(qall[:, n, 1, :], qraw[:, n, 1, :], t0_[:], OP.add)


                # ---- KV + attend, software-pipelined across segments:
                # the K/V projection + stats + normalize/rope of segment s+1
                # is emitted BEFORE the attends of segment s, so the PE's
                # attend matmuls (which wait on the tanh/exp chain) always
                # have the next segment's projection matmuls to overlap with,
                # and vice versa. ----
                kvctx = ExitStack()
                ap1 = kvctx.enter_context(tc.tile_pool(name="ap1", bufs=2))
                ap2 = kvctx.enter_context(tc.tile_pool(name="ap2", bufs=2))
                krw = kvctx.enter_context(tc.tile_pool(name="krw", bufs=2))
                ktp = kvctx.enter_context(tc.tile_pool(name="ktp", bufs=8))
                vtp = kvctx.enter_context(tc.tile_pool(name="vtp", bufs=8))
                epp = kvctx.enter_context(tc.tile_pool(name="epp", bufs=3))
                wks = kvctx.enter_context(tc.tile_pool(name="wk_s", bufs=2))

                def seg_ranges(seg):
                    """Per key-chunk query ranges implied by the sliding
                    window + causality (chunk-granular; the 0/1 bias handles
                    the exact interior masking). Ordered so the first range
                    covers the union (its psum `start` zeroes everything the
                    later sub-range matmuls accumulate into)."""
                    rngs = []
                    for st in range(SEG_CH):
                        o0 = seg * SEG + st * CH - OWN_OFF
                        q_lo = max(0, o0)
                        q_hi = min(TQ, o0 + CH - 1 + WINDOW)
                        if q_lo < q_hi:
                            rngs.append((st, q_lo, q_hi))
                    rngs.sort(key=lambda r: r[1] - r[2])
                    assert rngs and all(r[1] >= rngs[0][1] and r[2] <= rngs[0][2]
                                        for r in rngs)
                    return rngs

                FULL_RANGES = [(st, 0, TQ) for st in range(SEG_CH)]

                def attend_seg(kts, vts, bias_seg, first, ranges=None, post=None):
                    """Attend all N heads of one segment, with a one-head
                    lookahead: the logits/tanh/exp of head n are emitted
                    before the AV matmuls of head n-1, hiding the softcap
                    chain latency behind the next head's PE work."""
                    ranges = ranges or FULL_RANGES
                    _, u0, u1 = ranges[0]

                    def flush_head(n_head, eps_):
                        v_seg = vts[n_head // G]
                        for hc in range(HCH):
                            op = ps_mm.tile([CH, TQ], F32, tag="mm", name="op")
                            for i, (st, q0, q1) in enumerate(ranges):
                                nc.tensor.matmul(
                                    op[:, q0:q1], v_seg[:, st, hc * CH:(hc + 1) * CH],
                                    eps_[i][:, :q1 - q0],
                                    start=(i == 0), stop=(i == len(ranges) - 1),
                                    skip_group_check=True)
                            if first:
                                nc.scalar.activation(out=Oall[:, n_head, hc, :],
                                                     in_=op[:], func=AF.Copy, scale=1.0)
                            else:
                                nc.vector.tensor_tensor(Oall[:, n_head, hc, u0:u1],
                                                        Oall[:, n_head, hc, u0:u1],
                                                        op[:, u0:u1], OP.add)
                        sp = ps_st.tile([1, TQ], F32, tag="stat", name="sp")
                        for i, (st, q0, q1) in enumerate(ranges):
                            nc.tensor.matmul(sp[:, q0:q1], ones_b[:, :1],
                                             eps_[i][:, :q1 - q0],
                                             start=(i == 0), stop=(i == len(ranges) - 1),
                                             skip_group_check=True)
                        if first:
                            nc.scalar.activation(out=sums[:, n_head, :], in_=sp[:],
                                                 func=AF.Copy, scale=1.0)
                        else:
                            nc.vector.tensor_tensor(sums[:, n_head, u0:u1],
                                                    sums[:, n_head, u0:u1],
                                                    sp[:, u0:u1], OP.add)
                        if post is not None:
                            post(n_head)

                    prev = None
                    for n_head in range(N):
                        kT_seg = kts[n_head // G]
                        eps_ = []
                        for st, q0, q1 in ranges:
                            nq = q1 - q0
                            lg = ps_mm.tile([CH, TQ], F32, tag="mm", name="lg")
                            for hc in range(HCH):
                                nc.tensor.matmul(
                                    lg[:, :nq], kT_seg[:, hc, st * CH:(st + 1) * CH],
                                    qall[:, n_head, hc, q0:q1],
                                    start=(hc == 0), stop=(hc == HCH - 1))
                            th = epp.tile([CH, TQ], F32R, tag="tanh", name="th", bufs=2)
                            nc.scalar.activation(out=th[:, :nq], in_=lg[:, :nq],
                                                 func=AF.Tanh, scale=1.0 / SOFT_CAP)
                            ep = epp.tile([CH, TQ], BF16, tag="expp", name="ep", bufs=8)
                            nc.scalar.activation(out=ep[:, :nq], in_=th[:, :nq],
                                                 func=AF.Exp, scale=SOFT_CAP)
                            nc.vector.tensor_tensor(ep[:, :nq], ep[:, :nq],
                                                    bias_seg[:, st, q0:q1], OP.mult)
                            eps_.append(ep)
                        if prev is not None:
                            flush_head(*prev)
                        prev = (n_head, eps_)
                    flush_head(*prev)

                def project_segment(seg, dma0):
                    """K/V projection + stats + normalize + rope for one
                    segment; returns (kts, vts, bias_seg) ready to attend."""
                    ssl = slice(seg * SEG, (seg + 1) * SEG)
                    if dma0 is not None:
                        xs, bias_seg = dma0
                    else:
                        xs = xss.tile([CH, DCH, SEG], F16, tag="xs", name="xs")
                        nc.scalar.dma_start(xs[:], d_x[seg])
                        bias_seg = bss.tile([CH, SEG_CH, TQ], BF16, tag="bias", name="bias_seg")
                        nc.scalar.dma_start(bias_seg[:], d_bias[seg])
                    cos_s = sc_t[:, 0, ssl]
                    sin_s = sc_t[:, 1, ssl]

                    kts, vts = [], []
                    krow = krw.tile([1, KH * SEG], F32R, tag="krow", name="krow")
                    vst = krw.tile([CH, KH, SEG_CH], F32, tag="vst", name="vst")
                    for kh in range(KH):
                        wk_t = wks.tile([CH, DCH, H], F16, tag="wkv", name="wk_t")
                        nc.scalar.dma_start(wk_t[:], d_wk[kh])
                        kT_seg = ktp.tile([CH, HCH, SEG], F16, tag="kt", name="kT_seg")
                        for hc in range(HCH):
                            kp = ps_mm.tile([CH, SEG], F32, tag="mm", name="kp")
                            for dc in range(DCH):
                                nc.tensor.matmul(kp[:],
                                                 wk_t[:, dc, hc * CH:(hc + 1) * CH],
                                                 xs[:, dc, :],
                                                 start=(dc == 0), stop=(dc == DCH - 1))
                            ksq = ap1.tile([CH, SEG], F16, tag="sq", name="ksq")
                            nc.scalar.activation(out=ksq[:], in_=kp[:], func=AF.Square, scale=1.0)
                            kst = ps_st.tile([1, SEG], F32, tag="stat", name="kst")
                            nc.tensor.matmul(kst[:], ones_h[:, :1], ksq[:],
                                             start=True, stop=True)
                            nc.vector.tensor_scalar_mul(kT_seg[:, hc, :], kp[:],
                                                        ks_t[:, hc:hc + 1])
                            if hc == 0:
                                nc.scalar.activation(out=krow[:, kh * SEG:(kh + 1) * SEG],
                                                     in_=kst[:], func=AF.Copy, scale=1.0)
                            else:
                                nc.vector.tensor_tensor(krow[:, kh * SEG:(kh + 1) * SEG],
                                                        krow[:, kh * SEG:(kh + 1) * SEG],
                                                        kst[:], OP.add)
                        kts.append(kT_seg)

                        wv_t = wks.tile([CH, DCH, H], F16, tag="wkv", name="wv_t")
                        nc.scalar.dma_start(wv_t[:], d_wv[kh])
                        v_seg = vtp.tile([CH, SEG_CH, H], BF16, tag="vt", name="v_seg")
                        for sp2 in range(2):
                            vp = ps_mm.tile([CH, 2, H], F32, tag="mm", name="vp")
                            for sl in range(2):
                                st = sp2 * 2 + sl
                                for dc in range(DCH):
                                    nc.tensor.matmul(vp[:, sl, :],
                                                     xs[:, dc, st * CH:(st + 1) * CH],
                                                     wv_t[:, dc, :],
                                                     start=(dc == 0), stop=(dc == DCH - 1))
                            vsq = ap1.tile([CH, 2, H], F16, tag="sq", name="vsq")
                            nc.scalar.activation(out=vsq[:], in_=vp[:], func=AF.Square, scale=1.0)
                            for sl in range(2):
                                st = sp2 * 2 + sl
                                nc.vector.reduce_sum(vst[:, kh, st:st + 1], vsq[:, sl, :],
                                                     axis=mybir.AxisListType.X)
                            nc.scalar.activation(out=v_seg[:, sp2 * 2:(sp2 + 1) * 2, :],
                                                 in_=vp[:], func=AF.Copy, scale=1.0)
                        vts.append(v_seg)

                    # -- one Ln/Exp pair per segment for all k and v stats --
                    nc.scalar.activation(out=krow[:], in_=krow[:], func=AF.Ln,
                                         bias=eps1[:], scale=1.0 / H)
                    nc.scalar.activation(out=vst[:], in_=vst[:], func=AF.Ln,
                                         bias=epsp[:], scale=1.0 / H)
                    nc.scalar.activation(out=krow[:], in_=krow[:], func=AF.Exp, scale=-0.5)
                    krstd = krow
                    nc.scalar.activation(out=vst[:], in_=vst[:], func=AF.Exp, scale=-0.5)
                    vrstd = vst

                    return kts, vts, bias_seg, krstd, vrstd, cos_s, sin_s

                def project_phase2(ph1, khs=range(KH)):
                    # normalize + rope (DVE + bck broadcasts on the PE, gated
                    # on the Ln/Exp chain -- emitted AFTER the previous
                    # segment's attends so those never wait on it)
                    kts, vts, _, krstd, vrstd, cos_s, sin_s = ph1
                    for kh in khs:
                        kT_seg, v_seg = kts[kh], vts[kh]
                        bck = bcast(ap2, krstd[:, kh * SEG:(kh + 1) * SEG], "k", dtype=F16)
                        for hc in range(HCH):
                            nc.vector.tensor_tensor(kT_seg[:, hc, :], kT_seg[:, hc, :],
                                                    bck[:], OP.mult)
                        t0_ = ap2.tile([CH, SEG], F16, tag="kt0", name="t0_")
                        t1_ = ap2.tile([CH, SEG], F16, tag="kt1", name="t1_")
                        nc.vector.tensor_tensor(t0_[:], kT_seg[:, 0, :], sin_s, OP.mult)
                        nc.vector.tensor_tensor(t1_[:], kT_seg[:, 1, :], sin_s, OP.mult)
                        nc.vector.tensor_tensor(kT_seg[:, 0, :], kT_seg[:, 0, :], cos_s, OP.mult)
                        nc.vector.tensor_tensor(kT_seg[:, 1, :], kT_seg[:, 1, :], cos_s, OP.mult)
                        nc.vector.tensor_tensor(kT_seg[:, 0, :], kT_seg[:, 0, :], t1_[:], OP.subtract)
                        nc.vector.tensor_tensor(kT_seg[:, 1, :], kT_seg[:, 1, :], t0_[:], OP.add)
                        for st in range(SEG_CH):
                            nc.vector.tensor_scalar_mul(v_seg[:, st, :], v_seg[:, st, :],
                                                        vrstd[:, kh, st:st + 1])

                def normalize_head(n_head):
                    # divide Oall by the softmax sum: PE broadcast + fast
                    # DVE reciprocal, emitted per head right after its last
                    # attend so attn_vec can start while later heads attend
                    bcp = ps_st.tile([CH, TQ], F32, tag="stat", name="bcp")
                    nc.tensor.matmul(bcp[:], ones_brow[:], sums[:, n_head, :],
                                     start=True, stop=True)
                    bc = ap2.tile([CH, TQ], F32, tag="bc_s", name="bc", bufs=1)
                    nc.vector.reciprocal_approx_fast(bc[:], bcp[:])
                    for hc in range(HCH):
                        nc.vector.tensor_tensor(Oall[:, n_head, hc, :],
                                                Oall[:, n_head, hc, :], bc[:], OP.mult)

                seg_order = [OWN_SEG] + [s for s in range(NSEG) if s != OWN_SEG]
                pend = None
                for idx, seg in enumerate(seg_order):
                    dma0 = (xs0, bias0) if idx == 0 else None
                    ph1 = project_segment(seg, dma0)
                    if pend is not None:
                        # normalize head group 0 first so the NEXT iteration's
                        # attends find it ready without queueing behind this
                        # iteration's attend DVE work
                        project_phase2(ph1, range(0, 1))
                        attend_seg(*pend)
                        project_phase2(ph1, range(1, KH))
                    else:
                        project_phase2(ph1)
                    pend = (ph1[0], ph1[1], ph1[2], idx == 0, seg_ranges(seg),
                            None)
                kts_p, vts_p, bias_p, first_p, rng_p, _ = pend
                attend_seg(kts_p, vts_p, bias_p, first_p, rng_p,
                           post=None if NSEG_X else normalize_head)

                for sx in range(NSEG_X):
                    bias_seg = bss.tile([CH, SEG_CH, TQ], BF16, tag="bias", name="bias_seg")
                    nc.sync.dma_start(bias_seg[:], d_biasc[sx])
                    kts_x, vts_x = [], []
                    for kh in range(KH):
                        kT_seg = ktp.tile([CH, HCH, SEG], F16, tag="kt", name="kT_seg")
                        nc.sync.dma_start(kT_seg[:],
                                          d_kc[kh][:, :, sx * SEG:(sx + 1) * SEG])
                        v_seg = vtp.tile([CH, SEG_CH, H], BF16, tag="vt", name="v_seg")
                        nc.sync.dma_start(v_seg[:], d_vc[kh, sx])
                        kts_x.append(kT_seg)
                        vts_x.append(v_seg)
                    attend_seg(kts_x, vts_x, bias_seg, False, None,
                               post=normalize_head if sx == NSEG_X - 1 else None)

                kvctx.close()

            # ---- normalize O (divide by softmax sum), then attn_vec ----
            with tc.tile_pool(name="avacc", bufs=1) as avaccp:
              with tc.tile_pool(name="avp", bufs=3) as avpool, \
                    tc.tile_pool(name="wav_s", bufs=8) as wavs:
                attn_acc = avaccp.tile([CH, DCH, TQ], F32)
                pa_stat = ps_st.tile([1, TQ], F32, tag="stat", name="pa_stat")
                for dcq in range(4):
                    wav_ts = []
                    for n_head in range(N):
                        wav_t = wavs.tile([CH, HCH, TQ], BF16, tag="wavf", name="wav_t")
                        nc.sync.dma_start(wav_t[:], d_wav[n_head, dcq])
                        wav_ts.append(wav_t)
                    for dcl in range(4):
                        dc = dcq * 4 + dcl
                        # two half-groups (heads 0-3 / 4-7): the first half's
                        # matmuls start as soon as 4 heads are normalized
                        avp_a = ps_mm.tile([CH, TQ], F32, tag="mm", name="avp_a")
                        i = 0
                        for n_head in range(N // 2):
                            for hc in range(HCH):
                                nc.tensor.matmul(avp_a[:],
                                                 wav_ts[n_head][:, hc, dcl * CH:(dcl + 1) * CH],
                                                 Oall[:, n_head, hc, :],
                                                 start=(i == 0), stop=(i == N - 1))
                                i += 1
                        avp_b = ps_mm.tile([CH, TQ], F32, tag="mm", name="avp_b")
                        i = 0
                        for n_head in range(N // 2, N):
                            for hc in range(HCH):
                                nc.tensor.matmul(avp_b[:],
                                                 wav_ts[n_head][:, hc, dcl * CH:(dcl + 1) * CH],
                                                 Oall[:, n_head, hc, :],
                                                 start=(i == 0), stop=(i == N - 1))
                                i += 1
                        nc.scalar.activation(out=attn_acc[:, dc, :], in_=avp_a[:],
                                             func=AF.Copy, scale=1.0)
                        nc.vector.tensor_tensor(attn_acc[:, dc, :], attn_acc[:, dc, :],
                                                avp_b[:], OP.add)
                        sqa = avpool.tile([CH, TQ], F32R, tag="sqa", name="sqa")
                        nc.scalar.activation(out=sqa[:], in_=attn_acc[:, dc, :],
                                             func=AF.Square, scale=1.0)
                        nc.tensor.matmul(pa_stat[:], ones[:, :1], sqa[:],
                                         start=(dc == 0), stop=(dc == DCH - 1))

              # ---- post-attn norm + residual; pre-ffw norm ----
              if True:
                attn_out = cd.tile([CH, DCH, TQ], F16)
                ffw_in = cd.tile([CH, DCH, TQ], BF16)
                with tc.tile_pool(name="phc", bufs=1) as pc1, \
                        tc.tile_pool(name="phc2", bufs=2) as pc2:
                    x_own = pc1.tile([CH, DCH, SEG], F16, tag="xown")
                    nc.sync.dma_start(x_own[:], d_x[OWN_SEG])
                    rstd_pa = row_rstd(pc2, pa_stat, 1.0 / D, "pa")
                    bc = bcast(pc2, rstd_pa, "pa")
                    pf_stat = ps_st.tile([1, TQ], F32, tag="stat", name="pf_stat")
                    for dc in range(DCH):
                        tt = pc2.tile([CH, TQ], F32, tag="catmp", name="tt")
                        nc.vector.scalar_tensor_tensor(tt[:], attn_acc[:, dc, :],
                                                       pa_t[:, dc:dc + 1], bc[:],
                                                       OP.mult, OP.mult)
                        nc.vector.scalar_tensor_tensor(attn_out[:, dc, :],
                                                       x_own[:, dc, :], float(skip),
                                                       tt[:], OP.mult, OP.add)
                        sqf = pc2.tile([CH, TQ], F32R, tag="sqf", name="sqf")
                        nc.scalar.activation(out=sqf[:], in_=attn_out[:, dc, :],
                                             func=AF.Square, scale=1.0)
                        nc.tensor.matmul(pf_stat[:], ones[:, :1], sqf[:],
                                         start=(dc == 0), stop=(dc == DCH - 1))
                    rstd_pf = row_rstd(pc2, pf_stat, 1.0 / D, "pf")
                    bc2 = bcast(pc2, rstd_pf, "pf")
                    for dc in range(DCH):
                        nc.vector.tensor_tensor(ffw_in[:, dc, :], attn_out[:, dc, :], bc2[:], OP.mult)


        # ==================================================================
        # FFN
        # ==================================================================
        if True:
            with tc.tile_pool(name="dp1", bufs=1) as dp1, \
                    tc.tile_pool(name="dp2", bufs=2) as dp2, \
                    tc.tile_pool(name="actp", bufs=FHALF) as actp, \
                    tc.tile_pool(name="wg_s", bufs=4) as wgs, \
                    tc.tile_pool(name="wl_s", bufs=4) as wls:
                ffw_acc = dp1.tile([CH, DCH, TQ], F32)
                of_stat = None
                for half in range(2):
                    acts = []
                    for fgl in range(FHALF):
                        fg = half * FHALF + fgl
                        act = actp.tile([CH, FPG, TQ], BF16, tag="act", name=f"act{fgl}")
                        for fc in range(FPG):
                            fglob = fg * FPG + fc
                            gp = []
                            for g01 in range(2):
                                wg_t = wgs.tile([CH, DCH, CH], BF16, tag="wg", name="wg_t")
                                nc.sync.dma_start(wg_t[:], d_wg[g01, fglob])
                                pg = ps_mm.tile([CH, TQ], F32, tag="mm", name=f"pg{g01}")
                                for dc in range(DCH):
                                    nc.tensor.matmul(pg[:], wg_t[:, dc, :], ffw_in[:, dc, :],
                                                     start=(dc == 0), stop=(dc == DCH - 1))
                                gp.append(pg)
                            gel = dp2.tile([CH, TQ], F32, tag="gel", name="gel")
                            nc.scalar.activation(out=gel[:], in_=gp[0][:],
                                                 func=AF.Gelu_apprx_tanh, scale=1.0)
                            nc.vector.tensor_tensor(act[:, fc, :], gel[:], gp[1][:], OP.mult)
                        acts.append(act)
                    for dcq in range(4):
                        wl_ts = []
                        for fgl in range(FHALF):
                            fg = half * FHALF + fgl
                            wl_t = wls.tile([CH, FPG, 4, CH], BF16, tag="wl", name="wl_t")
                            nc.sync.dma_start(wl_t[:], d_wl[fg, dcq])
                            wl_ts.append(wl_t)
                        for dcl in range(4):
                            dc = dcq * 4 + dcl
                            pf = ps_mm.tile([CH, TQ], F32, tag="mm", name="pf")
                            i = 0
                            for fgl in range(FHALF):
                                for fc in range(FPG):
                                    nc.tensor.matmul(pf[:], wl_ts[fgl][:, fc, dcl, :],
                                                     acts[fgl][:, fc, :],
                                                     start=(i == 0), stop=(i == FHALF * FPG - 1))
                                    i += 1
                            if half == 0:
                                nc.scalar.activation(out=ffw_acc[:, dc, :], in_=pf[:],
                                                     func=AF.Copy, scale=1.0)
                            else:
                                nc.vector.tensor_tensor(ffw_acc[:, dc, :], ffw_acc[:, dc, :],
                                                        pf[:], OP.add)
                                sqo = dp2.tile([CH, TQ], F32R, tag="sqo", name="sqo")
                                nc.scalar.activation(out=sqo[:], in_=ffw_acc[:, dc, :],
                                                     func=AF.Square, scale=1.0)
                                if of_stat is None:
                                    of_stat = ps_st.tile([1, TQ], F32, tag="stat", name="of_stat")
                                nc.tensor.matmul(of_stat[:], ones[:, :1], sqo[:],
                                                 start=(dc == 0), stop=(dc == DCH - 1))
                # post-ffw norm + final residual (into attn_out, then store)
                rstd_of = row_rstd(dp2, of_stat, 1.0 / D, "of")
                bc3 = bcast(dp2, rstd_of, "of")
                for dc in range(DCH):
                    tt = dp2.tile([CH, TQ], F32, tag="fftmp", name="tt")
                    nc.vector.scalar_tensor_tensor(tt[:], ffw_acc[:, dc, :],
                                                   pf_t[:, dc:dc + 1], bc3[:],
                                                   OP.mult, OP.mult)
                    nc.vector.tensor_tensor(attn_out[:, dc, :], attn_out[:, dc, :], tt[:], OP.add)
                nc.sync.dma_start(d_out[:], attn_out[:])

    nc.finalize()
    return nc


_NC_CACHE = {}


def _in_maps(plan, w):
    in_maps = []
    for c in range(N_CORES):
        e = plan['per_core'][c]
        m = dict(x_t=e['x_t'], sc_t=e['sc_t'], bias_t=e['bias_t'],
                 wq_t=w['wq_t'], wk_t=w['wk_t'], wv_t=w['wv_t'],
                 wav_t=w['wav_t'], wg_t=w['wg_t'], wl_t=w['wl_t'],
                 q_scale=w['q_scale'], k_scale=w['k_scale'],
                 post_attn=w['post_attn'], post_ffw=w['post_ffw'])
        if plan['EXTRA']:
            m.update(kc_t=e['kc_t'], vc_t=e['vc_t'], biasc_t=e['biasc_t'])
        in_maps.append(m)
    return in_maps


def _nc_key(plan, w):
    return (plan['W'], plan['OWN_OFF'], plan['EXTRA'], w['skip'])


def kernel(**inputs) -> np.ndarray:
    from concourse.bass_utils import run_bass_kernel_spmd

    plan = _plan(inputs)
    w = _prep_weights(inputs)
    key = _nc_key(plan, w)
    if key not in _NC_CACHE:
        _NC_CACHE[key] = _build_nc(*key)
    nc = _NC_CACHE[key]

    res = run_bass_kernel_spmd(nc, _in_maps(plan, w), core_ids=list(range(N_CORES)))

    out = np.zeros((B, T, D), np.float32)
    for c in range(N_CORES):
        e = plan['per_core'][c]
        o = res.results[c]['out_t']          # [128, DCH, TQ]
        out[e['b'], e['t0']:e['t0'] + TQ] = o.transpose(2, 1, 0).reshape(TQ, D)
    return out

